# revision 1
# baseline (speedup 1.0000x reference)
"""Self-contained multi-head attention kernel for 8 Trainium2 NeuronCores.

Problem: x[8,1024,768] -> fused qkv proj -> 12-head attention (n=1024,
d_head=64) -> out proj + bias. Data-parallel over batch: core b handles x[b].

All device layouts are chosen so no on-device transposes are needed:
  - host supplies x transposed (xT [768,1024])
  - phase 1a computes [q;k]^T [1536,1024] (w stationary, xT moving)
  - phase 1b computes v_aug [1024, 12*65] (xT stationary, w_v moving),
    with a ones-column per head (rank-1 K=1 matmul) so the PV matmul
    also yields softmax row-sums
  - dots produce S^T[j,i] per head; K=64 head pairs are row-packed into
    the 128-row PE array; exp runs on ScalarE with the 1/sqrt(d) scale
    folded into the activation's free affine
  - PV uses v_aug as stationary (M=65): rows 0..63 = unnormalized out^T,
    row 64 = row-sum r; normalize via reciprocal + K=1 broadcast matmul
    + vector multiply
  - out projection uses A^T tiles directly as lhsT; bias via K=1 ones
    matmul against b_out
"""
import numpy as np

import concourse.bass as bass
import concourse.mybir as mybir
import concourse.tile as tile
from concourse import bacc

# The activation-table insertion pass picks sets greedily, which thrashes
# between exp_and_others and natural_log (~2.7us per reload) when a kernel
# interleaves Exp and Ln. Restrict Exp/Ln coverage to the combined set so
# one load serves the whole kernel. Set ids are positional, so entries are
# edited in place, never removed.
if not hasattr(bacc, "_orig_get_act_tables"):
    bacc._orig_get_act_tables = bacc.get_activation_tables


def _combined_act_tables(arch):
    tables = dict(bacc._orig_get_act_tables(arch))
    exp = mybir.ActivationFunctionType.Exp
    ln = mybir.ActivationFunctionType.Ln
    both = tables.get("natural_log_exp_and_others")
    if not both or exp not in both or ln not in both:
        return tables  # combined set unavailable: keep default behavior
    for name, funcs in tables.items():
        if name != "natural_log_exp_and_others" and (exp in funcs or ln in funcs):
            tables[name] = funcs - {exp, ln}
    return tables


bacc.get_activation_tables = _combined_act_tables

DIM = 768
HEADS = 12
DH = 64
INNER = HEADS * DH
N_TOK = 1024
BATCH = 8
N_CORES = 8
SCALE = DH ** -0.5
VW = HEADS * (DH + 1)  # 780: v columns with per-head ones column

F32 = mybir.dt.float32
MM_DT = mybir.dt.float32r  # matmul operand dtype


def _nsplits(total, cap=512):
    out = []
    s = 0
    while s < total:
        e = min(s + cap, total)
        out.append((s, e))
        s = e
    return out


def build_nc(n_tok=N_TOK, num_devices=N_CORES, mm_dt=MM_DT, debug=False,
             phases=(1, 2, 3), pairs=HEADS // 2,
             qkp_bufs=6, exps_bufs=9, psa_bufs=2, pv_bufs=2, lookahead=2,
             reps=1, usb_copy=True, fused=True, ablag=6, skip_norm=False,
             qk_copy_act=True, bcast_gpsimd=False, bias_dve=True,
             fused_ab=True, lagi=6, fill_even=True, norm_split=False,
             p3_deep=True):
    nc = bacc.Bacc("TRN2", target_bir_lowering=False, debug=debug,
                   num_devices=num_devices)
    nt = n_tok // 128          # token tiles (i and j)
    KT = DIM // 128            # contraction tiles over model dim
    ET = (2 * DIM) // 128      # q+k rows -> 12 tiles
    ATT = INNER // 128         # A^T e-tiles -> 6

    # inputs are declared in the matmul dtype so the plain DMA load
    # satisfies the "rounded to FP32r" producer rule
    xT_d = nc.dram_tensor("xT", (DIM, n_tok), mm_dt, kind="ExternalInput")
    wqk_d = nc.dram_tensor("wqk", (DIM, 2 * DIM), mm_dt, kind="ExternalInput")
    wv_d = nc.dram_tensor("wv", (DIM, VW), mm_dt, kind="ExternalInput")
    vind_d = nc.dram_tensor("vind", (1, VW), mm_dt, kind="ExternalInput")
    ones_d = nc.dram_tensor("ones", (1, 128), mm_dt, kind="ExternalInput")
    wout_d = nc.dram_tensor("wout", (INNER, DIM), mm_dt, kind="ExternalInput")
    bout_d = nc.dram_tensor("bout", (1, DIM), mm_dt, kind="ExternalInput")
    o_d = nc.dram_tensor("o", (n_tok, DIM), F32, kind="ExternalOutput")

    use2 = 2 in phases
    npairs = pairs if use2 else 0

    with tile.TileContext(nc) as tc:
        with (
            tc.tile_pool(name="persist", bufs=1) as pp,
            tc.tile_pool(name="qkp", bufs=qkp_bufs) as qkp,
            tc.tile_pool(name="exps", bufs=exps_bufs) as exps,
            tc.tile_pool(name="p2s", bufs=1) as p2s,
            tc.tile_pool(name="psa", bufs=psa_bufs, space="PSUM") as psa,
            tc.tile_pool(name="pspv", bufs=pv_bufs, space="PSUM") as pspv,
        ):
            v_aug = pp.tile([128, nt, VW], mm_dt)    # v + ones cols, by i-tile
            a_T = pp.tile([128, ATT, n_tok], mm_dt)  # normalized attn out ^T
            smalls = pp.tile([1, 128 + VW + DIM], mm_dt)
            ones = smalls[:, 0:128]
            vind = smalls[:, 128:128 + VW]
            bout = smalls[:, 128 + VW:]
            import contextlib as _ctl
            _loop = (tc.For_i(0, reps, 1,
                              hint_engines=(mybir.EngineType.PE,
                                            mybir.EngineType.Activation,
                                            mybir.EngineType.DVE))
                     if reps > 1 else _ctl.nullcontext())
            with _loop:
                nc.sync.dma_start(ones, ones_d.ap())
                nc.sync.dma_start(vind, vind_d.ap())
                nc.sync.dma_start(bout, bout_d.ap())

                qk_tiles = {}

                with tc.tile_pool(name="p1", bufs=1) as p1:
                    # ---------------- phase 1: qkv projections ----------------
                    if 1 in phases:
                        xt = p1.tile([128, KT, n_tok], mm_dt)
                        wv = p1.tile([128, KT, VW], mm_dt)
                        wqk_t = [p1.tile([128, KT, 256], mm_dt, name=f"wqk{pr}")
                                 for pr in range(ET // 2)]
                        # interleave xt / pair-0,1 qk chunks at k-tile
                        # granularity so the first 1a chains pipeline with DMA;
                        # then wv (1b rides as filler inside pair 0), then the
                        # remaining qk pairs
                        def load_wqk_chunk(pr, kt):
                            nc.sync.dma_start(
                                wqk_t[pr][:, kt, :],
                                wqk_d.ap()[kt * 128:(kt + 1) * 128,
                                           pr * 256:(pr + 1) * 256])
                        if fused:
                            for kt in range(KT):
                                nc.sync.dma_start(
                                    xt[:, kt, :],
                                    xT_d.ap()[kt * 128:(kt + 1) * 128, :])
                                load_wqk_chunk(0, kt)
                                load_wqk_chunk(1, kt)
                            for kt in range(KT):
                                nc.sync.dma_start(
                                    wv[:, kt, :],
                                    wv_d.ap()[kt * 128:(kt + 1) * 128, :])
                            for pr in range(2, ET // 2):
                                for kt in range(KT):
                                    load_wqk_chunk(pr, kt)
                        else:
                            # sequential driver runs 1b first: stream xt+wv
                            # so 1b chains pipeline with the DMA, then the
                            # qk pairs in consumption order
                            for kt in range(KT):
                                nc.sync.dma_start(
                                    xt[:, kt, :],
                                    xT_d.ap()[kt * 128:(kt + 1) * 128, :])
                                nc.sync.dma_start(
                                    wv[:, kt, :],
                                    wv_d.ap()[kt * 128:(kt + 1) * 128, :])
                            for pr in range(ET // 2):
                                for kt in range(KT):
                                    load_wqk_chunk(pr, kt)

                    def emit_1b_one(it):
                        # v_aug[i, c] = sum_d xT[d, i] * wv[d, c]  (+ ones cols)
                        pv_ = psa.tile([128, 1024], F32, tag="psa")
                        for kt in range(KT):
                            for (s, e) in _nsplits(VW):
                                nc.tensor.matmul(
                                    pv_[:, s:e],
                                    xt[:, kt, it * 128:(it + 1) * 128],
                                    wv[:, kt, s:e],
                                    start=(kt == 0), stop=False,
                                )
                        for (s, e) in _nsplits(VW):
                            nc.tensor.matmul(
                                pv_[:, s:e], ones[0:1, 0:128], vind[0:1, s:e],
                                start=False, stop=True,
                            )
                        nc.vector.tensor_copy(v_aug[:, it, :], pv_[:, 0:VW])

                    def emit_1b():
                        for it in range(nt):
                            emit_1b_one(it)

                    def emit_1a_one(mt):
                        # qkT[e, i] = sum_d wqk[d, e] * xT[d, i], one 128-row tile
                        if True:
                            pq = psa.tile([128, 1024], F32, tag="psa")
                            wpr, wo = (mt, 0) if mt < ET // 2 else (mt - ET // 2, 128)
                            for kt in range(KT):
                                for (s, e) in _nsplits(n_tok):
                                    nc.tensor.matmul(
                                        pq[:, s:e],
                                        wqk_t[wpr][:, kt, wo:wo + 128],
                                        xt[:, kt, s:e],
                                        start=(kt == 0), stop=(kt == KT - 1),
                                    )
                            qt = qkp.tile([128, n_tok], mm_dt, tag="qk",
                                          name=f"qk{mt}")
                            nc.vector.tensor_copy(qt[:], pq[:, 0:n_tok])
                            qk_tiles[mt] = qt

                    def emit_1a_pair(pr):
                        for mt in (pr, ET // 2 + pr):
                            emit_1a_one(mt)

                    # --- phase 2: one unit = one head. Global modulo software
                    # pipeline: every step emits one dots (PE+ACT producer), an
                    # optional filler (1b / 1a-prefetch chain), and pops one
                    # deferred consumer (PV matmul or normalization) from a
                    # global queue that trails LAG steps behind, so PE never
                    # waits on the exp of the tile it just produced and the
                    # work mix per step is uniform across unit boundaries.
                    LAG = min(ablag, nt)
                    workq = []

                    def qstep(force=False):
                        while workq and (force or len(workq) > LAG):
                            workq.pop(0)()

                    def emit_unit(pair, half, fillers):
                        qt = qk_tiles[pair]
                        kt_ = qk_tiles[ET // 2 + pair]
                        h = 2 * pair + half
                        p0 = half * 64
                        up = pspv.tile([65, 1024], F32, tag="pv", name=f"up{h}")
                        ets = {}

                        def dots(jt):
                            ps = psa.tile([128, 1024], F32, tag="psa")
                            for (s, e) in _nsplits(n_tok):
                                nc.tensor.matmul(
                                    ps[:, s:e],
                                    kt_[p0:p0 + 64, jt * 128:(jt + 1) * 128],
                                    qt[p0:p0 + 64, s:e],
                                    start=True, stop=True,
                                )
                            et = exps.tile([128, n_tok], mm_dt, tag="expS",
                                           name=f"et{half}_{jt}")
                            nc.scalar.activation(
                                et[:], ps[:, 0:n_tok],
                                mybir.ActivationFunctionType.Exp, scale=SCALE)
                            ets[jt] = et

                        def pv(jt):
                            for (s, e) in _nsplits(n_tok):
                                nc.tensor.matmul(
                                    up[:, s:e],
                                    v_aug[:, jt, h * 65:h * 65 + 65],
                                    ets[jt][:, s:e],
                                    start=(jt == 0), stop=(jt == nt - 1),
                                )
                            del ets[jt]

                        def norm():
                            # a_T[h rows] = up[0:64] * (1 / up[64]).
                            # 1/r via exp(-ln r) on ScalarE: a [1, n] DVE
                            # reciprocal runs on one lane at 8 cyc/elem
                            # (~8.5 us) and would serialize the pipeline.
                            lnr = p2s.tile([1, n_tok], F32, tag="lnr")
                            nc.scalar.activation(
                                lnr[:], up[64:65, 0:n_tok],
                                mybir.ActivationFunctionType.Ln)
                            rinv = p2s.tile([1, n_tok], mm_dt, tag="rinv")
                            nc.scalar.activation(
                                rinv[:], lnr[:],
                                mybir.ActivationFunctionType.Exp, scale=-1.0)
                            if usb_copy:
                                usb = p2s.tile([64, n_tok], F32, tag="usb")
                                nc.vector.tensor_copy(usb[:], up[0:64, 0:n_tok])
                                mul_in = usb[:]
                            else:
                                mul_in = up[0:64, 0:n_tok]
                            bc = psa.tile([128, 1024], F32, tag="psa")
                            for (s, e) in _nsplits(n_tok):
                                nc.tensor.matmul(
                                    bc[0:64, s:e], ones[0:1, 0:64],
                                    rinv[0:1, s:e], start=True, stop=True,
                                )
                            nc.vector.tensor_mul(
                                a_T[p0:p0 + 64, h // 2, :], mul_in,
                                bc[0:64, 0:n_tok])

                        for jt in range(nt):
                            dots(jt)
                            if fillers:
                                fillers.pop(0)()
                            workq.append(lambda jt=jt: pv(jt))
                            qstep()
                        while fillers:
                            fillers.pop(0)()
                        workq.append(norm)

                    def emit_pair_ab(pair):
                        # both heads of a pair interleaved per j-tile:
                        # alternating PE row groups overlap fill/drain, and
                        # exp feeds stay dense. PSUM: 2 dots tiles + 2 up
                        # accumulators = 8 banks.
                        qt = qk_tiles[pair]
                        kt_ = qk_tiles[ET // 2 + pair]
                        ups = {}
                        etsd = {0: {}, 1: {}}
                        for half in (0, 1):
                            ups[half] = pspv.tile([65, 1024], F32, tag="pv",
                                                  name=f"upab{half}")

                        def dots(half, jt):
                            p0 = half * 64
                            ps = psa.tile([128, 1024], F32, tag="psa")
                            for (s, e) in _nsplits(n_tok):
                                nc.tensor.matmul(
                                    ps[:, s:e],
                                    kt_[p0:p0 + 64, jt * 128:(jt + 1) * 128],
                                    qt[p0:p0 + 64, s:e],
                                    start=True, stop=True,
                                )
                            et = exps.tile([128, n_tok], mm_dt, tag="expS",
                                           name=f"etab{half}_{jt}")
                            nc.scalar.activation(
                                et[:], ps[:, 0:n_tok],
                                mybir.ActivationFunctionType.Exp, scale=SCALE)
                            etsd[half][jt] = et

                        def pv(half, jt):
                            h = 2 * pair + half
                            for (s, e) in _nsplits(n_tok):
                                nc.tensor.matmul(
                                    ups[half][:, s:e],
                                    v_aug[:, jt, h * 65:h * 65 + 65],
                                    etsd[half][jt][:, s:e],
                                    start=(jt == 0), stop=(jt == nt - 1),
                                )
                            del etsd[half][jt]

                        def norm(half):
                            h = 2 * pair + half
                            p0 = half * 64
                            up = ups[half]
                            lnr = p2s.tile([1, n_tok], F32, tag="lnr")
                            nc.scalar.activation(
                                lnr[:], up[64:65, 0:n_tok],
                                mybir.ActivationFunctionType.Ln)
                            rinv = p2s.tile([1, n_tok], mm_dt, tag="rinv")
                            nc.scalar.activation(
                                rinv[:], lnr[:],
                                mybir.ActivationFunctionType.Exp, scale=-1.0)
                            usb = p2s.tile([64, n_tok], F32, tag="usb")
                            nc.vector.tensor_copy(usb[:], up[0:64, 0:n_tok])
                            dst = a_T[p0:p0 + 64, h // 2, :]
                            if bcast_gpsimd:
                                # broadcast 1/r on the otherwise-idle GPSIMD,
                                # then multiply in place on DVE
                                nc.gpsimd.partition_broadcast(
                                    dst.bitcast(F32), rinv.bitcast(F32)[:])
                                nc.vector.tensor_mul(dst, dst.bitcast(F32),
                                                     usb[:])
                            else:
                                bc = psa.tile([128, 1024], F32, tag="psa")
                                for (s, e) in _nsplits(n_tok):
                                    nc.tensor.matmul(
                                        bc[0:64, s:e], ones[0:1, 0:64],
                                        rinv[0:1, s:e], start=True, stop=True,
                                    )
                                nc.vector.tensor_mul(
                                    dst, usb[:], bc[0:64, 0:n_tok])

                        ABLAG = ablag
                        for jt in range(nt):
                            dots(0, jt)
                            dots(1, jt)
                            if jt >= ABLAG:
                                pv(0, jt - ABLAG)
                                pv(1, jt - ABLAG)
                        for jt in range(max(nt - ABLAG, 0), nt):
                            pv(0, jt)
                            pv(1, jt)
                        if not skip_norm:
                            norm(0)
                            norm(1)

                    def emit_pair_fused(pair, fillers, lagi=6):
                        # both halves interleaved per j-tile (row-group
                        # alternation overlaps PE fill/drain) while keeping
                        # the fused filler structure. PSUM: 2 dots tiles +
                        # 2 up accumulators = 8 banks.
                        qt = qk_tiles[pair]
                        kt_ = qk_tiles[ET // 2 + pair]
                        ups = {}
                        for half in (0, 1):
                            ups[half] = pspv.tile([65, 1024], F32, tag="pv",
                                                  name=f"upf{half}")
                        ets = {}

                        def dots(half, jt):
                            p0 = half * 64
                            ps = psa.tile([128, 1024], F32, tag="psa")
                            for (s, e) in _nsplits(n_tok):
                                nc.tensor.matmul(
                                    ps[:, s:e],
                                    kt_[p0:p0 + 64, jt * 128:(jt + 1) * 128],
                                    qt[p0:p0 + 64, s:e],
                                    start=True, stop=True,
                                )
                            et = exps.tile([128, n_tok], mm_dt, tag="expS",
                                           name=f"etf{half}_{jt}")
                            nc.scalar.activation(
                                et[:], ps[:, 0:n_tok],
                                mybir.ActivationFunctionType.Exp, scale=SCALE)
                            ets[(half, jt)] = et

                        def pv(half, jt):
                            h = 2 * pair + half
                            for (s, e) in _nsplits(n_tok):
                                nc.tensor.matmul(
                                    ups[half][:, s:e],
                                    v_aug[:, jt, h * 65:h * 65 + 65],
                                    ets[(half, jt)][:, s:e],
                                    start=(jt == 0), stop=(jt == nt - 1),
                                )
                            del ets[(half, jt)]

                        rinvs = {}

                        def norm_act(half):
                            # 1/r on ScalarE only; emitted 1-2 queue steps
                            # before the PE bcast so the ln->exp chain never
                            # head-of-line-blocks the PE stream
                            up = ups[half]
                            lnr = p2s.tile([1, n_tok], F32, tag="lnr")
                            nc.scalar.activation(
                                lnr[:], up[64:65, 0:n_tok],
                                mybir.ActivationFunctionType.Ln)
                            rinv = p2s.tile([1, n_tok], mm_dt,
                                            tag=(f"rinv{half}" if norm_split
                                                 else "rinv"),
                                            name=f"rinvf{half}")
                            nc.scalar.activation(
                                rinv[:], lnr[:],
                                mybir.ActivationFunctionType.Exp, scale=-1.0)
                            rinvs[half] = rinv

                        def norm_rest(half):
                            h = 2 * pair + half
                            p0 = half * 64
                            up = ups[half]
                            rinv = rinvs[half]
                            usb = p2s.tile([64, n_tok], F32, tag="usb")
                            nc.vector.tensor_copy(usb[:], up[0:64, 0:n_tok])
                            bc = psa.tile([128, 1024], F32, tag="psa")
                            for (s, e) in _nsplits(n_tok):
                                nc.tensor.matmul(
                                    bc[0:64, s:e], ones[0:1, 0:64],
                                    rinv[0:1, s:e], start=True, stop=True,
                                )
                            nc.vector.tensor_mul(
                                a_T[p0:p0 + 64, h // 2, :], usb[:],
                                bc[0:64, 0:n_tok])

                        for jt in range(nt):
                            dots(0, jt)
                            dots(1, jt)
                            if fillers:
                                fillers.pop(0)()
                            workq.append(lambda jt=jt: pv(0, jt))
                            workq.append(lambda jt=jt: pv(1, jt))
                            while len(workq) > lagi:
                                workq.pop(0)()
                        while fillers:
                            fillers.pop(0)()
                        if norm_split:
                            workq.append(lambda: norm_act(0))
                            workq.append(lambda: norm_act(1))
                            workq.append(lambda: norm_rest(0))
                            workq.append(lambda: norm_rest(1))
                        else:
                            workq.append(lambda: (norm_act(0), norm_rest(0)))
                            workq.append(lambda: (norm_act(1), norm_rest(1)))

                    # software-pipelined emission driver
                    if 1 in phases and use2:
                        emit_1a_pair(0)
                        emit_1a_pair(1)
                        # filler queues: 1b chains ride inside pair 0; 1a
                        # prefetch for pair pr rides inside pair pr-2, half B
                        fill = {}
                        for pair in range(npairs):
                            for half in (0, 1):
                                fill[(pair, half)] = []
                        for it in range(nt):
                            u = (0, 0) if it < 6 else (0, 1)
                            fill[u].append(lambda it=it: emit_1b_one(it))
                        for pr in range(2, ET // 2):
                            host = (pr - 2, 1)
                            if host not in fill:
                                host = (npairs - 1, 1)
                            fill[host].append(lambda m=pr: emit_1a_one(m))
                            fill[host].append(
                                lambda m=ET // 2 + pr: emit_1a_one(m))
                        if fused_ab:
                            if fill_even:
                                # re-spread: 1b 0..5 in pair 0; 1b 6,7 +
                                # 1a(2) in pair 1; 1a(pr) in pair pr-2
                                fl = {p: [] for p in range(npairs)}
                                fl[0] = [lambda it=it: emit_1b_one(it)
                                         for it in range(min(6, nt))]
                                if npairs > 1:
                                    fl[1] = ([lambda it=it: emit_1b_one(it)
                                              for it in range(6, nt)] +
                                             [lambda: emit_1a_one(2),
                                              lambda: emit_1a_one(ET // 2 + 2)])
                                for pr in range(3, ET // 2):
                                    host = min(pr - 2, npairs - 1)
                                    fl[host].append(
                                        lambda m=pr: emit_1a_one(m))
                                    fl[host].append(
                                        lambda m=ET // 2 + pr: emit_1a_one(m))
                                for pair in range(npairs):
                                    emit_pair_fused(pair, fl[pair], lagi=lagi)
                            else:
                                for pair in range(npairs):
                                    emit_pair_fused(
                                        pair,
                                        fill[(pair, 0)] + fill[(pair, 1)],
                                        lagi=lagi)
                        else:
                            for pair in range(npairs):
                                for half in (0, 1):
                                    emit_unit(pair, half, fill[(pair, half)])
                        qstep(force=True)
                    elif 1 in phases:
                        emit_1b()
                        for pr in range(ET // 2):
                            emit_1a_pair(pr)
                    else:
                        for pair in range(npairs):
                            for half in (0, 1):
                                emit_unit(pair, half, [])
                        qstep(force=True)

                # ------------- phase 3: output projection + bias -------------
                with (
                    tc.tile_pool(name="pw", bufs=1) as pw,
                    tc.tile_pool(name="p3o", bufs=3) as p3o,
                ):
                    if 3 in phases:
                        wout = pw.tile([128, ATT, DIM], mm_dt)
                        for kt in range(ATT):
                            nc.sync.dma_start(
                                wout[:, kt, :],
                                wout_d.ap()[kt * 128:(kt + 1) * 128, :])
                        if bias_dve:
                            # bias broadcast built once; the i-tile loop adds
                            # it on the otherwise-idle DVE instead of 16 K=1
                            # matmuls on the (binding) PE
                            bias_bc = pw.tile([128, DIM], F32)
                            bps = psa.tile([128, 1024], F32, tag="psa")
                            for (s, e) in _nsplits(DIM):
                                nc.tensor.matmul(
                                    bps[0:128, s:e], ones[0:1, 0:128],
                                    bout[0:1, s:e], start=True, stop=True,
                                )
                            nc.scalar.copy(bias_bc[:], bps[:, 0:DIM])
                        for it in range(nt):
                            if p3_deep and it % 2 == 1:
                                # the PV accumulator banks are idle in p3:
                                # alternate output accumulators across both
                                # pools for a deeper pipeline
                                po = pspv.tile([128, 1024], F32, tag="pv",
                                               name="po_b")
                            else:
                                po = psa.tile([128, 1024], F32, tag="psa")
                            for kt in range(ATT):
                                for (s, e) in _nsplits(DIM):
                                    nc.tensor.matmul(
                                        po[:, s:e],
                                        a_T[:, kt, it * 128:(it + 1) * 128],
                                        wout[:, kt, s:e],
                                        start=(kt == 0),
                                        stop=(bias_dve and kt == ATT - 1),
                                    )
                            if not bias_dve:
                                for (s, e) in _nsplits(DIM):
                                    nc.tensor.matmul(
                                        po[:, s:e], ones[0:1, 0:128],
                                        bout[0:1, s:e],
                                        start=False, stop=True,
                                    )
                            osb = p3o.tile([128, DIM], F32, tag="osb")
                            if bias_dve:
                                nc.vector.tensor_add(osb[:], po[:, 0:DIM],
                                                     bias_bc[:])
                            else:
                                nc.scalar.copy(osb[:], po[:, 0:DIM])
                            nc.sync.dma_start(
                                o_d.ap()[it * 128:(it + 1) * 128, :], osb[:])


    nc.compile()
    return nc


def host_prep(x, w_qkv, w_out, b_out, batch=BATCH):
    """Build per-core input maps from the full problem inputs."""
    x = np.asarray(x, dtype=np.float32)
    w_qkv = np.asarray(w_qkv, dtype=np.float32)
    w_out = np.asarray(w_out, dtype=np.float32)
    b_out = np.asarray(b_out, dtype=np.float32)

    w_q = w_qkv[:, 0:INNER]
    w_k = w_qkv[:, INNER:2 * INNER]
    w_v = w_qkv[:, 2 * INNER:3 * INNER]
    wqk = np.zeros((DIM, 2 * INNER), dtype=np.float32)
    for p in range(HEADS // 2):
        wqk[:, p * 256:p * 256 + 128] = w_q[:, p * 128:(p + 1) * 128]
        wqk[:, p * 256 + 128:(p + 1) * 256] = w_k[:, p * 128:(p + 1) * 128]
    wv = np.zeros((DIM, VW), dtype=np.float32)
    vind = np.zeros((1, VW), dtype=np.float32)
    for h in range(HEADS):
        wv[:, h * 65:h * 65 + 64] = w_v[:, h * 64:(h + 1) * 64]
        vind[0, h * 65 + 64] = 1.0
    shared = {
        "ones": np.ones((1, 128), dtype=np.float32),
        "wqk": wqk,
        "wv": wv,
        "vind": vind,
        "wout": np.ascontiguousarray(w_out),
        "bout": np.ascontiguousarray(b_out.reshape(1, DIM)),
    }
    in_maps = []
    for b in range(batch):
        m = dict(shared)
        m["xT"] = np.ascontiguousarray(x[b].T)
        in_maps.append(m)
    return in_maps


# --- inline PJRT runner (build once, call many) ---
def _make_runner(nc, n_cores):
    import jax
    from jax.sharding import Mesh, PartitionSpec
    from jax.experimental.shard_map import shard_map
    from concourse import bass2jax

    bass2jax.install_neuronx_cc_hook()
    partition_name = nc.partition_id_tensor.name if nc.partition_id_tensor else None
    in_names, out_names, out_avals, zero_outs = [], [], [], []
    for alloc in nc.m.functions[0].allocations:
        if not isinstance(alloc, mybir.MemoryLocationSet):
            continue
        name = alloc.memorylocations[0].name
        if alloc.kind == "ExternalInput":
            if name != partition_name:
                in_names.append(name)
        elif alloc.kind == "ExternalOutput":
            shape = tuple(alloc.tensor_shape)
            dtype = mybir.dt.np(alloc.dtype)
            out_names.append(name)
            out_avals.append(jax.core.ShapedArray(shape, dtype))
            zero_outs.append(np.zeros(shape, dtype))
    n_params = len(in_names)
    n_outs = len(out_avals)
    all_in_names = list(in_names) + list(out_names)
    if partition_name is not None:
        all_in_names.append(partition_name)

    def _body(*args):
        operands = list(args)
        if partition_name is not None:
            operands.append(bass2jax.partition_id_tensor())
        outs = bass2jax._bass_exec_p.bind(
            *operands,
            out_avals=tuple(out_avals),
            in_names=tuple(all_in_names),
            out_names=tuple(out_names),
            lowering_input_output_aliases=(),
            sim_require_finite=True,
            sim_require_nnan=True,
            nc=nc,
        )
        return tuple(outs)

    donate = tuple(range(n_params, n_params + n_outs))
    if n_cores == 1:
        fn = jax.jit(_body, donate_argnums=donate, keep_unused=True)

        def run(in_maps):
            args = [np.asarray(in_maps[0][n]) for n in in_names]
            out_arrs = fn(*args, *[z.copy() for z in zero_outs])
            jax.block_until_ready(out_arrs)
            return [{n: np.asarray(out_arrs[i]) for i, n in enumerate(out_names)}]
        return run

    devices = jax.devices()[:n_cores]
    mesh = Mesh(np.asarray(devices), ("core",))
    in_specs = (PartitionSpec("core"),) * (n_params + n_outs)
    out_specs = (PartitionSpec("core"),) * n_outs
    fn = jax.jit(
        shard_map(_body, mesh=mesh, in_specs=in_specs, out_specs=out_specs,
                  check_rep=False),
        donate_argnums=donate, keep_unused=True,
    )

    def run(in_maps):
        per_core = [[np.asarray(m[n]) for n in in_names] for m in in_maps]
        concat_in = [
            np.concatenate([per_core[c][i] for c in range(n_cores)], axis=0)
            for i in range(n_params)
        ]
        concat_zeros = [
            np.zeros((n_cores * z.shape[0], *z.shape[1:]), z.dtype)
            for z in zero_outs
        ]
        out_arrs = fn(*concat_in, *concat_zeros)
        jax.block_until_ready(out_arrs)
        return [
            {n: np.asarray(out_arrs[i]).reshape(n_cores, *out_avals[i].shape)[c]
             for i, n in enumerate(out_names)}
            for c in range(n_cores)
        ]
    return run


_CACHE = {}


def get_runner():
    if "run" not in _CACHE:
        nc = build_nc()
        _CACHE["nc"] = nc
        _CACHE["run"] = _make_runner(nc, N_CORES)
    return _CACHE["run"]


def kernel(x, w_qkv, w_out, b_out):
    run = get_runner()
    in_maps = host_prep(x, w_qkv, w_out, b_out)
    res = run(in_maps)
    return np.stack([res[b]["o"] for b in range(BATCH)], axis=0)



# revision 13
# speedup vs baseline: 1.2126x; 1.2126x over previous
"""Self-contained multi-head attention kernel for 8 Trainium2 NeuronCores.

Problem: x[8,1024,768] -> fused qkv proj -> 12-head attention (n=1024,
d_head=64) -> out proj + bias. Data-parallel over batch: core b handles x[b].

All device layouts are chosen so no on-device transposes are needed:
  - host supplies x transposed (xT [768,1024])
  - phase 1a computes [q;k]^T [1536,1024] (w stationary, xT moving)
  - phase 1b computes v_aug [1024, 12*65] (xT stationary, w_v moving),
    with a ones-column per head (rank-1 K=1 matmul) so the PV matmul
    also yields softmax row-sums
  - dots produce S^T[j,i] per head; K=64 head pairs are row-packed into
    the 128-row PE array; exp runs on ScalarE with the 1/sqrt(d) scale
    folded into the activation's free affine
  - PV uses v_aug as stationary (M=65): rows 0..63 = unnormalized out^T,
    row 64 = row-sum r; normalize via reciprocal + K=1 broadcast matmul
    + vector multiply
  - out projection uses A^T tiles directly as lhsT; bias via K=1 ones
    matmul against b_out
"""
import numpy as np

import concourse.bass as bass
import concourse.mybir as mybir
import concourse.tile as tile
from concourse import bacc

# The activation-table insertion pass picks sets greedily, which thrashes
# between exp_and_others and natural_log (~2.7us per reload) when a kernel
# interleaves Exp and Ln. Restrict Exp/Ln coverage to the combined set so
# one load serves the whole kernel. Set ids are positional, so entries are
# edited in place, never removed.
if not hasattr(bacc, "_orig_get_act_tables"):
    bacc._orig_get_act_tables = bacc.get_activation_tables


def _combined_act_tables(arch):
    tables = dict(bacc._orig_get_act_tables(arch))
    exp = mybir.ActivationFunctionType.Exp
    ln = mybir.ActivationFunctionType.Ln
    both = tables.get("natural_log_exp_and_others")
    if not both or exp not in both or ln not in both:
        return tables  # combined set unavailable: keep default behavior
    for name, funcs in tables.items():
        if name != "natural_log_exp_and_others" and (exp in funcs or ln in funcs):
            tables[name] = funcs - {exp, ln}
    return tables


bacc.get_activation_tables = _combined_act_tables

DIM = 768
HEADS = 12
DH = 64
INNER = HEADS * DH
N_TOK = 1024
BATCH = 8
N_CORES = 8
SCALE = DH ** -0.5
VW = HEADS * (DH + 1)  # 780: v columns with per-head ones column

F32 = mybir.dt.float32
MM_DT = mybir.dt.float16  # matmul operand dtype (2-byte: 1024-col moving ops)


_NSPLIT_CAP = 512


def _nsplits(total, cap=None):
    if cap is None:
        cap = _NSPLIT_CAP
    out = []
    s = 0
    while s < total:
        e = min(s + cap, total)
        out.append((s, e))
        s = e
    return out


def build_nc(n_tok=N_TOK, num_devices=N_CORES, mm_dt=MM_DT, debug=False,
             phases=(1, 2, 3), pairs=HEADS // 2,
             qkp_bufs=6, exps_bufs=9, psa_bufs=2, pv_bufs=2, lookahead=2,
             reps=1, usb_copy=True, fused=True, ablag=6, skip_norm=False,
             qk_copy_act=True, bcast_gpsimd=False, bias_dve=True,
             fused_ab=True, lagi=6, fill_even=True, norm_split=False,
             p3_deep=True, cap=512, norm_pool=1):
    global _NSPLIT_CAP
    _NSPLIT_CAP = cap
    nc = bacc.Bacc("TRN2", target_bir_lowering=False, debug=debug,
                   num_devices=num_devices)
    nt = n_tok // 128          # token tiles (i and j)
    KT = DIM // 128            # contraction tiles over model dim
    ET = (2 * DIM) // 128      # q+k rows -> 12 tiles
    ATT = INNER // 128         # A^T e-tiles -> 6

    # inputs are declared in the matmul dtype so the plain DMA load
    # satisfies the "rounded to FP32r" producer rule
    xT_d = nc.dram_tensor("xT", (DIM, n_tok), mm_dt, kind="ExternalInput")
    wqk_d = nc.dram_tensor("wqk", (DIM, 2 * DIM), mm_dt, kind="ExternalInput")
    wv_d = nc.dram_tensor("wv", (DIM, VW), mm_dt, kind="ExternalInput")
    vind_d = nc.dram_tensor("vind", (1, VW), mm_dt, kind="ExternalInput")
    ones_d = nc.dram_tensor("ones", (1, 128), mm_dt, kind="ExternalInput")
    wout_d = nc.dram_tensor("wout", (INNER, DIM), mm_dt, kind="ExternalInput")
    bout_d = nc.dram_tensor("bout", (1, DIM), mm_dt, kind="ExternalInput")
    o_d = nc.dram_tensor("o", (n_tok, DIM), F32, kind="ExternalOutput")

    use2 = 2 in phases
    npairs = pairs if use2 else 0

    with tile.TileContext(nc) as tc:
        with (
            tc.tile_pool(name="persist", bufs=1) as pp,
            tc.tile_pool(name="qkp", bufs=qkp_bufs) as qkp,
            tc.tile_pool(name="exps", bufs=exps_bufs) as exps,
            tc.tile_pool(name="p2s", bufs=1) as p2s,
            tc.tile_pool(name="psa", bufs=psa_bufs, space="PSUM") as psa,
            tc.tile_pool(name="pspv", bufs=pv_bufs, space="PSUM") as pspv,
        ):
            v_aug = pp.tile([128, nt, VW], mm_dt)    # v + ones cols, by i-tile
            a_T = pp.tile([128, ATT, n_tok], mm_dt)  # normalized attn out ^T
            smalls = pp.tile([1, 128 + VW + DIM], mm_dt)
            ones = smalls[:, 0:128]
            vind = smalls[:, 128:128 + VW]
            bout = smalls[:, 128 + VW:]
            import contextlib as _ctl
            _loop = (tc.For_i(0, reps, 1,
                              hint_engines=(mybir.EngineType.PE,
                                            mybir.EngineType.Activation,
                                            mybir.EngineType.DVE))
                     if reps > 1 else _ctl.nullcontext())
            with _loop:
                nc.sync.dma_start(ones, ones_d.ap())
                nc.sync.dma_start(vind, vind_d.ap())
                nc.sync.dma_start(bout, bout_d.ap())

                qk_tiles = {}

                with tc.tile_pool(name="p1", bufs=1) as p1:
                    # ---------------- phase 1: qkv projections ----------------
                    if 1 in phases:
                        xt = p1.tile([128, KT, n_tok], mm_dt)
                        wv = p1.tile([128, KT, VW], mm_dt)
                        wqk_t = [p1.tile([128, KT, 256], mm_dt, name=f"wqk{pr}")
                                 for pr in range(ET // 2)]
                        # interleave xt / pair-0,1 qk chunks at k-tile
                        # granularity so the first 1a chains pipeline with DMA;
                        # then wv (1b rides as filler inside pair 0), then the
                        # remaining qk pairs
                        def load_wqk_chunk(pr, kt):
                            nc.sync.dma_start(
                                wqk_t[pr][:, kt, :],
                                wqk_d.ap()[kt * 128:(kt + 1) * 128,
                                           pr * 256:(pr + 1) * 256])
                        if fused:
                            for kt in range(KT):
                                nc.sync.dma_start(
                                    xt[:, kt, :],
                                    xT_d.ap()[kt * 128:(kt + 1) * 128, :])
                                load_wqk_chunk(0, kt)
                                load_wqk_chunk(1, kt)
                            for kt in range(KT):
                                nc.sync.dma_start(
                                    wv[:, kt, :],
                                    wv_d.ap()[kt * 128:(kt + 1) * 128, :])
                            for pr in range(2, ET // 2):
                                for kt in range(KT):
                                    load_wqk_chunk(pr, kt)
                        else:
                            # sequential driver runs 1b first: stream xt+wv
                            # so 1b chains pipeline with the DMA, then the
                            # qk pairs in consumption order
                            for kt in range(KT):
                                nc.sync.dma_start(
                                    xt[:, kt, :],
                                    xT_d.ap()[kt * 128:(kt + 1) * 128, :])
                                nc.sync.dma_start(
                                    wv[:, kt, :],
                                    wv_d.ap()[kt * 128:(kt + 1) * 128, :])
                            for pr in range(ET // 2):
                                for kt in range(KT):
                                    load_wqk_chunk(pr, kt)

                    def emit_1b_one(it):
                        # v_aug[i, c] = sum_d xT[d, i] * wv[d, c]  (+ ones cols)
                        pv_ = psa.tile([128, 1024], F32, tag="psa")
                        for kt in range(KT):
                            for (s, e) in _nsplits(VW):
                                nc.tensor.matmul(
                                    pv_[:, s:e],
                                    xt[:, kt, it * 128:(it + 1) * 128],
                                    wv[:, kt, s:e],
                                    start=(kt == 0), stop=False,
                                )
                        for (s, e) in _nsplits(VW):
                            nc.tensor.matmul(
                                pv_[:, s:e], ones[0:1, 0:128], vind[0:1, s:e],
                                start=False, stop=True,
                            )
                        nc.vector.tensor_copy(v_aug[:, it, :], pv_[:, 0:VW])

                    def emit_1b():
                        for it in range(nt):
                            emit_1b_one(it)

                    def emit_1a_one(mt):
                        # qkT[e, i] = sum_d wqk[d, e] * xT[d, i], one 128-row tile
                        if True:
                            pq = psa.tile([128, 1024], F32, tag="psa")
                            wpr, wo = (mt, 0) if mt < ET // 2 else (mt - ET // 2, 128)
                            for kt in range(KT):
                                for (s, e) in _nsplits(n_tok):
                                    nc.tensor.matmul(
                                        pq[:, s:e],
                                        wqk_t[wpr][:, kt, wo:wo + 128],
                                        xt[:, kt, s:e],
                                        start=(kt == 0), stop=(kt == KT - 1),
                                    )
                            qt = qkp.tile([128, n_tok], mm_dt, tag="qk",
                                          name=f"qk{mt}")
                            nc.vector.tensor_copy(qt[:], pq[:, 0:n_tok])
                            qk_tiles[mt] = qt

                    def emit_1a_pair(pr):
                        for mt in (pr, ET // 2 + pr):
                            emit_1a_one(mt)

                    # --- phase 2: one unit = one head. Global modulo software
                    # pipeline: every step emits one dots (PE+ACT producer), an
                    # optional filler (1b / 1a-prefetch chain), and pops one
                    # deferred consumer (PV matmul or normalization) from a
                    # global queue that trails LAG steps behind, so PE never
                    # waits on the exp of the tile it just produced and the
                    # work mix per step is uniform across unit boundaries.
                    LAG = min(ablag, nt)
                    workq = []

                    def qstep(force=False):
                        while workq and (force or len(workq) > LAG):
                            workq.pop(0)()

                    def emit_unit(pair, half, fillers):
                        qt = qk_tiles[pair]
                        kt_ = qk_tiles[ET // 2 + pair]
                        h = 2 * pair + half
                        p0 = half * 64
                        up = pspv.tile([65, 1024], F32, tag="pv", name=f"up{h}")
                        ets = {}

                        def dots(jt):
                            ps = psa.tile([128, 1024], F32, tag="psa")
                            for (s, e) in _nsplits(n_tok):
                                nc.tensor.matmul(
                                    ps[:, s:e],
                                    kt_[p0:p0 + 64, jt * 128:(jt + 1) * 128],
                                    qt[p0:p0 + 64, s:e],
                                    start=True, stop=True,
                                )
                            et = exps.tile([128, n_tok], mm_dt, tag="expS",
                                           name=f"et{half}_{jt}")
                            nc.scalar.activation(
                                et[:], ps[:, 0:n_tok],
                                mybir.ActivationFunctionType.Exp, scale=SCALE)
                            ets[jt] = et

                        def pv(jt):
                            for (s, e) in _nsplits(n_tok):
                                nc.tensor.matmul(
                                    up[:, s:e],
                                    v_aug[:, jt, h * 65:h * 65 + 65],
                                    ets[jt][:, s:e],
                                    start=(jt == 0), stop=(jt == nt - 1),
                                )
                            del ets[jt]

                        def norm():
                            # a_T[h rows] = up[0:64] * (1 / up[64]).
                            # 1/r via exp(-ln r) on ScalarE: a [1, n] DVE
                            # reciprocal runs on one lane at 8 cyc/elem
                            # (~8.5 us) and would serialize the pipeline.
                            lnr = p2s.tile([1, n_tok], F32, tag="lnr")
                            nc.scalar.activation(
                                lnr[:], up[64:65, 0:n_tok],
                                mybir.ActivationFunctionType.Ln)
                            rinv = p2s.tile([1, n_tok], mm_dt, tag="rinv")
                            nc.scalar.activation(
                                rinv[:], lnr[:],
                                mybir.ActivationFunctionType.Exp, scale=-1.0)
                            if usb_copy:
                                usb = p2s.tile([64, n_tok], F32, tag="usb")
                                nc.vector.tensor_copy(usb[:], up[0:64, 0:n_tok])
                                mul_in = usb[:]
                            else:
                                mul_in = up[0:64, 0:n_tok]
                            bc = psa.tile([128, 1024], F32, tag="psa")
                            for (s, e) in _nsplits(n_tok):
                                nc.tensor.matmul(
                                    bc[0:64, s:e], ones[0:1, 0:64],
                                    rinv[0:1, s:e], start=True, stop=True,
                                )
                            nc.vector.tensor_mul(
                                a_T[p0:p0 + 64, h // 2, :], mul_in,
                                bc[0:64, 0:n_tok])

                        for jt in range(nt):
                            dots(jt)
                            if fillers:
                                fillers.pop(0)()
                            workq.append(lambda jt=jt: pv(jt))
                            qstep()
                        while fillers:
                            fillers.pop(0)()
                        workq.append(norm)

                    def emit_pair_ab(pair):
                        # both heads of a pair interleaved per j-tile:
                        # alternating PE row groups overlap fill/drain, and
                        # exp feeds stay dense. PSUM: 2 dots tiles + 2 up
                        # accumulators = 8 banks.
                        qt = qk_tiles[pair]
                        kt_ = qk_tiles[ET // 2 + pair]
                        ups = {}
                        etsd = {0: {}, 1: {}}
                        for half in (0, 1):
                            ups[half] = pspv.tile([65, 1024], F32, tag="pv",
                                                  name=f"upab{half}")

                        def dots(half, jt):
                            p0 = half * 64
                            ps = psa.tile([128, 1024], F32, tag="psa")
                            for (s, e) in _nsplits(n_tok):
                                nc.tensor.matmul(
                                    ps[:, s:e],
                                    kt_[p0:p0 + 64, jt * 128:(jt + 1) * 128],
                                    qt[p0:p0 + 64, s:e],
                                    start=True, stop=True,
                                )
                            et = exps.tile([128, n_tok], mm_dt, tag="expS",
                                           name=f"etab{half}_{jt}")
                            nc.scalar.activation(
                                et[:], ps[:, 0:n_tok],
                                mybir.ActivationFunctionType.Exp, scale=SCALE)
                            etsd[half][jt] = et

                        def pv(half, jt):
                            h = 2 * pair + half
                            for (s, e) in _nsplits(n_tok):
                                nc.tensor.matmul(
                                    ups[half][:, s:e],
                                    v_aug[:, jt, h * 65:h * 65 + 65],
                                    etsd[half][jt][:, s:e],
                                    start=(jt == 0), stop=(jt == nt - 1),
                                )
                            del etsd[half][jt]

                        def norm(half):
                            h = 2 * pair + half
                            p0 = half * 64
                            up = ups[half]
                            lnr = p2s.tile([1, n_tok], F32, tag="lnr")
                            nc.scalar.activation(
                                lnr[:], up[64:65, 0:n_tok],
                                mybir.ActivationFunctionType.Ln)
                            rinv = p2s.tile([1, n_tok], mm_dt, tag="rinv")
                            nc.scalar.activation(
                                rinv[:], lnr[:],
                                mybir.ActivationFunctionType.Exp, scale=-1.0)
                            usb = p2s.tile([64, n_tok], F32, tag="usb")
                            nc.vector.tensor_copy(usb[:], up[0:64, 0:n_tok])
                            dst = a_T[p0:p0 + 64, h // 2, :]
                            if bcast_gpsimd:
                                # broadcast 1/r on the otherwise-idle GPSIMD,
                                # then multiply in place on DVE
                                nc.gpsimd.partition_broadcast(
                                    dst.bitcast(F32), rinv.bitcast(F32)[:])
                                nc.vector.tensor_mul(dst, dst.bitcast(F32),
                                                     usb[:])
                            else:
                                bc = psa.tile([128, 1024], F32, tag="psa")
                                for (s, e) in _nsplits(n_tok):
                                    nc.tensor.matmul(
                                        bc[0:64, s:e], ones[0:1, 0:64],
                                        rinv[0:1, s:e], start=True, stop=True,
                                    )
                                nc.vector.tensor_mul(
                                    dst, usb[:], bc[0:64, 0:n_tok])

                        ABLAG = ablag
                        for jt in range(nt):
                            dots(0, jt)
                            dots(1, jt)
                            if jt >= ABLAG:
                                pv(0, jt - ABLAG)
                                pv(1, jt - ABLAG)
                        for jt in range(max(nt - ABLAG, 0), nt):
                            pv(0, jt)
                            pv(1, jt)
                        if not skip_norm:
                            norm(0)
                            norm(1)

                    def emit_pair_fused(pair, fillers, lagi=6):
                        # both halves interleaved per j-tile (row-group
                        # alternation overlaps PE fill/drain) while keeping
                        # the fused filler structure. PSUM: 2 dots tiles +
                        # 2 up accumulators = 8 banks.
                        qt = qk_tiles[pair]
                        kt_ = qk_tiles[ET // 2 + pair]
                        ups = {}
                        for half in (0, 1):
                            ups[half] = pspv.tile([65, 1024], F32, tag="pv",
                                                  name=f"upf{half}")
                        ets = {}

                        def dots(half, jt):
                            p0 = half * 64
                            ps = psa.tile([128, 1024], F32, tag="psa")
                            for (s, e) in _nsplits(n_tok):
                                nc.tensor.matmul(
                                    ps[:, s:e],
                                    kt_[p0:p0 + 64, jt * 128:(jt + 1) * 128],
                                    qt[p0:p0 + 64, s:e],
                                    start=True, stop=True,
                                )
                            et = exps.tile([128, n_tok], mm_dt, tag="expS",
                                           name=f"etf{half}_{jt}")
                            nc.scalar.activation(
                                et[:], ps[:, 0:n_tok],
                                mybir.ActivationFunctionType.Exp, scale=SCALE)
                            ets[(half, jt)] = et

                        def pv(half, jt):
                            h = 2 * pair + half
                            for (s, e) in _nsplits(n_tok):
                                nc.tensor.matmul(
                                    ups[half][:, s:e],
                                    v_aug[:, jt, h * 65:h * 65 + 65],
                                    ets[(half, jt)][:, s:e],
                                    start=(jt == 0), stop=(jt == nt - 1),
                                )
                            del ets[(half, jt)]

                        rinvs = {}
                        usbs = {}

                        def norm_act(half):
                            up = ups[half]
                            if norm_pool:
                                # copy out^T + r row to SBUF (frees the PSUM
                                # bank in one op), 1/r approx on DVE: no ACT
                                # table pressure, no [1,n] ln/exp lane waste
                                usb = p2s.tile([65, n_tok], F32,
                                               tag=f"usb{half}",
                                               name=f"usbf{half}")
                                nc.vector.tensor_copy(usb[:],
                                                      up[0:65, 0:n_tok])
                                rinv = p2s.tile([1, n_tok], F32,
                                                tag=f"rinv{half}",
                                                name=f"rinvf{half}")
                                if norm_pool == 3:
                                    lnr = p2s.tile([1, n_tok], F32,
                                                   tag=f"lnr{half}")
                                    nc.scalar.activation(
                                        lnr[:], usb[64:65, :],
                                        mybir.ActivationFunctionType.Ln)
                                    nc.scalar.activation(
                                        rinv[:], lnr[:],
                                        mybir.ActivationFunctionType.Exp,
                                        scale=-1.0)
                                else:
                                    # custom DVE / gpsimd ISA ops read the
                                    # memloc's partition 0 regardless of the
                                    # AP offset: stage the r row (partition
                                    # 64) to a partition-0 tile via an
                                    # SBUF->SBUF DMA (off-engine, idle in
                                    # phase 2) before the DVE reciprocal
                                    rrow = p2s.tile([1, n_tok], F32,
                                                    tag=f"rrow{half}",
                                                    name=f"rrowf{half}")
                                    nc.sync.dma_start(rrow[:],
                                                      usb[64:65, :])
                                    nc.vector.reciprocal_approx_fast(
                                        rinv[:], rrow[:])
                                usbs[half] = usb
                                rinvs[half] = rinv
                                return
                            lnr = p2s.tile([1, n_tok], F32, tag="lnr")
                            nc.scalar.activation(
                                lnr[:], up[64:65, 0:n_tok],
                                mybir.ActivationFunctionType.Ln)
                            rinv = p2s.tile([1, n_tok], mm_dt,
                                            tag=(f"rinv{half}" if norm_split
                                                 else "rinv"),
                                            name=f"rinvf{half}")
                            nc.scalar.activation(
                                rinv[:], lnr[:],
                                mybir.ActivationFunctionType.Exp, scale=-1.0)
                            rinvs[half] = rinv

                        def norm_rest(half):
                            h = 2 * pair + half
                            p0 = half * 64
                            up = ups[half]
                            rinv = rinvs[half]
                            if norm_pool:
                                # broadcast 1/r on the idle Pool/GPSIMD,
                                # multiply on DVE
                                usb = usbs[half]
                                if norm_pool == 2:
                                    rin16 = p2s.tile([1, n_tok], mm_dt,
                                                     tag=f"ri16{half}")
                                    nc.vector.tensor_copy(rin16[:], rinv[:])
                                    bcp = psa.tile([128, 1024], F32,
                                                   tag="psa")
                                    for (s, e) in _nsplits(n_tok):
                                        nc.tensor.matmul(
                                            bcp[0:64, s:e], ones[0:1, 0:64],
                                            rin16[0:1, s:e],
                                            start=True, stop=True)
                                    nc.vector.tensor_mul(
                                        a_T[p0:p0 + 64, h // 2, :],
                                        usb[0:64, :], bcp[0:64, 0:n_tok])
                                    return
                                bc = p2s.tile([64, n_tok], F32,
                                              tag=f"bc{half}",
                                              name=f"bcf{half}")
                                nc.gpsimd.partition_broadcast(bc[:], rinv[:])
                                nc.vector.tensor_mul(
                                    a_T[p0:p0 + 64, h // 2, :],
                                    usb[0:64, :], bc[:])
                                return
                            usb = p2s.tile([64, n_tok], F32, tag="usb")
                            nc.vector.tensor_copy(usb[:], up[0:64, 0:n_tok])
                            bc = psa.tile([128, 1024], F32, tag="psa")
                            for (s, e) in _nsplits(n_tok):
                                nc.tensor.matmul(
                                    bc[0:64, s:e], ones[0:1, 0:64],
                                    rinv[0:1, s:e], start=True, stop=True,
                                )
                            nc.vector.tensor_mul(
                                a_T[p0:p0 + 64, h // 2, :], usb[:],
                                bc[0:64, 0:n_tok])

                        for jt in range(nt):
                            dots(0, jt)
                            dots(1, jt)
                            if fillers:
                                fillers.pop(0)()
                            workq.append(lambda jt=jt: pv(0, jt))
                            workq.append(lambda jt=jt: pv(1, jt))
                            while len(workq) > lagi:
                                workq.pop(0)()
                        while fillers:
                            fillers.pop(0)()
                        if norm_split:
                            workq.append(lambda: norm_act(0))
                            workq.append(lambda: norm_act(1))
                            workq.append(lambda: norm_rest(0))
                            workq.append(lambda: norm_rest(1))
                        else:
                            workq.append(lambda: (norm_act(0), norm_rest(0)))
                            workq.append(lambda: (norm_act(1), norm_rest(1)))

                    # software-pipelined emission driver
                    if 1 in phases and use2:
                        emit_1a_pair(0)
                        emit_1a_pair(1)
                        # filler queues: 1b chains ride inside pair 0; 1a
                        # prefetch for pair pr rides inside pair pr-2, half B
                        fill = {}
                        for pair in range(npairs):
                            for half in (0, 1):
                                fill[(pair, half)] = []
                        for it in range(nt):
                            u = (0, 0) if it < 6 else (0, 1)
                            fill[u].append(lambda it=it: emit_1b_one(it))
                        for pr in range(2, ET // 2):
                            host = (pr - 2, 1)
                            if host not in fill:
                                host = (npairs - 1, 1)
                            fill[host].append(lambda m=pr: emit_1a_one(m))
                            fill[host].append(
                                lambda m=ET // 2 + pr: emit_1a_one(m))
                        if fused_ab:
                            if fill_even:
                                # re-spread: 1b 0..5 in pair 0; 1b 6,7 +
                                # 1a(2) in pair 1; 1a(pr) in pair pr-2
                                fl = {p: [] for p in range(npairs)}
                                fl[0] = [lambda it=it: emit_1b_one(it)
                                         for it in range(min(6, nt))]
                                if npairs > 1:
                                    fl[1] = ([lambda it=it: emit_1b_one(it)
                                              for it in range(6, nt)] +
                                             [lambda: emit_1a_one(2),
                                              lambda: emit_1a_one(ET // 2 + 2)])
                                for pr in range(3, ET // 2):
                                    host = min(pr - 2, npairs - 1)
                                    fl[host].append(
                                        lambda m=pr: emit_1a_one(m))
                                    fl[host].append(
                                        lambda m=ET // 2 + pr: emit_1a_one(m))
                                for pair in range(npairs):
                                    emit_pair_fused(pair, fl[pair], lagi=lagi)
                            else:
                                for pair in range(npairs):
                                    emit_pair_fused(
                                        pair,
                                        fill[(pair, 0)] + fill[(pair, 1)],
                                        lagi=lagi)
                        else:
                            for pair in range(npairs):
                                for half in (0, 1):
                                    emit_unit(pair, half, fill[(pair, half)])
                        qstep(force=True)
                    elif 1 in phases:
                        emit_1b()
                        for pr in range(ET // 2):
                            emit_1a_pair(pr)
                    else:
                        for pair in range(npairs):
                            for half in (0, 1):
                                emit_unit(pair, half, [])
                        qstep(force=True)

                # ------------- phase 3: output projection + bias -------------
                with (
                    tc.tile_pool(name="pw", bufs=1) as pw,
                    tc.tile_pool(name="p3o", bufs=3) as p3o,
                ):
                    if 3 in phases:
                        wout = pw.tile([128, ATT, DIM], mm_dt)
                        for kt in range(ATT):
                            nc.sync.dma_start(
                                wout[:, kt, :],
                                wout_d.ap()[kt * 128:(kt + 1) * 128, :])
                        if bias_dve:
                            # bias broadcast built once; the i-tile loop adds
                            # it on the otherwise-idle DVE instead of 16 K=1
                            # matmuls on the (binding) PE
                            bias_bc = pw.tile([128, DIM], F32)
                            bps = psa.tile([128, 1024], F32, tag="psa")
                            for (s, e) in _nsplits(DIM):
                                nc.tensor.matmul(
                                    bps[0:128, s:e], ones[0:1, 0:128],
                                    bout[0:1, s:e], start=True, stop=True,
                                )
                            nc.scalar.copy(bias_bc[:], bps[:, 0:DIM])
                        for it in range(nt):
                            if p3_deep and it % 2 == 1:
                                # the PV accumulator banks are idle in p3:
                                # alternate output accumulators across both
                                # pools for a deeper pipeline
                                po = pspv.tile([128, 1024], F32, tag="pv",
                                               name="po_b")
                            else:
                                po = psa.tile([128, 1024], F32, tag="psa")
                            for kt in range(ATT):
                                for (s, e) in _nsplits(DIM):
                                    nc.tensor.matmul(
                                        po[:, s:e],
                                        a_T[:, kt, it * 128:(it + 1) * 128],
                                        wout[:, kt, s:e],
                                        start=(kt == 0),
                                        stop=(bias_dve and kt == ATT - 1),
                                    )
                            if not bias_dve:
                                for (s, e) in _nsplits(DIM):
                                    nc.tensor.matmul(
                                        po[:, s:e], ones[0:1, 0:128],
                                        bout[0:1, s:e],
                                        start=False, stop=True,
                                    )
                            osb = p3o.tile([128, DIM], F32, tag="osb")
                            if bias_dve:
                                nc.vector.tensor_add(osb[:], po[:, 0:DIM],
                                                     bias_bc[:])
                            else:
                                nc.scalar.copy(osb[:], po[:, 0:DIM])
                            nc.sync.dma_start(
                                o_d.ap()[it * 128:(it + 1) * 128, :], osb[:])


    nc.compile()
    return nc


def host_prep(x, w_qkv, w_out, b_out, batch=BATCH):
    """Build per-core input maps from the full problem inputs."""
    x = np.asarray(x, dtype=np.float32)
    w_qkv = np.asarray(w_qkv, dtype=np.float32)
    w_out = np.asarray(w_out, dtype=np.float32)
    b_out = np.asarray(b_out, dtype=np.float32)

    w_q = w_qkv[:, 0:INNER]
    w_k = w_qkv[:, INNER:2 * INNER]
    w_v = w_qkv[:, 2 * INNER:3 * INNER]
    wqk = np.zeros((DIM, 2 * INNER), dtype=np.float32)
    for p in range(HEADS // 2):
        wqk[:, p * 256:p * 256 + 128] = w_q[:, p * 128:(p + 1) * 128]
        wqk[:, p * 256 + 128:(p + 1) * 256] = w_k[:, p * 128:(p + 1) * 128]
    wv = np.zeros((DIM, VW), dtype=np.float32)
    vind = np.zeros((1, VW), dtype=np.float32)
    for h in range(HEADS):
        wv[:, h * 65:h * 65 + 64] = w_v[:, h * 64:(h + 1) * 64]
        vind[0, h * 65 + 64] = 1.0
    hd = np.float16  # matmul operand dtype on device (must match MM_DT)
    shared = {
        "ones": np.ones((1, 128), dtype=hd),
        "wqk": wqk.astype(hd),
        "wv": wv.astype(hd),
        "vind": vind.astype(hd),
        "wout": np.ascontiguousarray(w_out).astype(hd),
        "bout": np.ascontiguousarray(b_out.reshape(1, DIM)).astype(hd),
    }
    in_maps = []
    for b in range(batch):
        m = dict(shared)
        m["xT"] = np.ascontiguousarray(x[b].T).astype(hd)
        in_maps.append(m)
    return in_maps


# --- inline PJRT runner (build once, call many) ---
def _make_runner(nc, n_cores):
    import jax
    from jax.sharding import Mesh, PartitionSpec
    from jax.experimental.shard_map import shard_map
    from concourse import bass2jax

    bass2jax.install_neuronx_cc_hook()
    partition_name = nc.partition_id_tensor.name if nc.partition_id_tensor else None
    in_names, out_names, out_avals, zero_outs = [], [], [], []
    for alloc in nc.m.functions[0].allocations:
        if not isinstance(alloc, mybir.MemoryLocationSet):
            continue
        name = alloc.memorylocations[0].name
        if alloc.kind == "ExternalInput":
            if name != partition_name:
                in_names.append(name)
        elif alloc.kind == "ExternalOutput":
            shape = tuple(alloc.tensor_shape)
            dtype = mybir.dt.np(alloc.dtype)
            out_names.append(name)
            out_avals.append(jax.core.ShapedArray(shape, dtype))
            zero_outs.append(np.zeros(shape, dtype))
    n_params = len(in_names)
    n_outs = len(out_avals)
    all_in_names = list(in_names) + list(out_names)
    if partition_name is not None:
        all_in_names.append(partition_name)

    def _body(*args):
        operands = list(args)
        if partition_name is not None:
            operands.append(bass2jax.partition_id_tensor())
        outs = bass2jax._bass_exec_p.bind(
            *operands,
            out_avals=tuple(out_avals),
            in_names=tuple(all_in_names),
            out_names=tuple(out_names),
            lowering_input_output_aliases=(),
            sim_require_finite=True,
            sim_require_nnan=True,
            nc=nc,
        )
        return tuple(outs)

    donate = tuple(range(n_params, n_params + n_outs))
    if n_cores == 1:
        fn = jax.jit(_body, donate_argnums=donate, keep_unused=True)

        def run(in_maps):
            args = [np.asarray(in_maps[0][n]) for n in in_names]
            out_arrs = fn(*args, *[z.copy() for z in zero_outs])
            jax.block_until_ready(out_arrs)
            return [{n: np.asarray(out_arrs[i]) for i, n in enumerate(out_names)}]
        return run

    devices = jax.devices()[:n_cores]
    mesh = Mesh(np.asarray(devices), ("core",))
    in_specs = (PartitionSpec("core"),) * (n_params + n_outs)
    out_specs = (PartitionSpec("core"),) * n_outs
    fn = jax.jit(
        shard_map(_body, mesh=mesh, in_specs=in_specs, out_specs=out_specs,
                  check_rep=False),
        donate_argnums=donate, keep_unused=True,
    )

    def run(in_maps):
        per_core = [[np.asarray(m[n]) for n in in_names] for m in in_maps]
        concat_in = [
            np.concatenate([per_core[c][i] for c in range(n_cores)], axis=0)
            for i in range(n_params)
        ]
        concat_zeros = [
            np.zeros((n_cores * z.shape[0], *z.shape[1:]), z.dtype)
            for z in zero_outs
        ]
        out_arrs = fn(*concat_in, *concat_zeros)
        jax.block_until_ready(out_arrs)
        return [
            {n: np.asarray(out_arrs[i]).reshape(n_cores, *out_avals[i].shape)[c]
             for i, n in enumerate(out_names)}
            for c in range(n_cores)
        ]
    return run


_CACHE = {}


def get_runner():
    if "run" not in _CACHE:
        nc = build_nc()
        _CACHE["nc"] = nc
        _CACHE["run"] = _make_runner(nc, N_CORES)
    return _CACHE["run"]


def kernel(x, w_qkv, w_out, b_out):
    run = get_runner()
    in_maps = host_prep(x, w_qkv, w_out, b_out)
    res = run(in_maps)
    return np.stack([res[b]["o"] for b in range(BATCH)], axis=0)



# revision 21
# speedup vs baseline: 1.2447x; 1.0265x over previous
"""Self-contained multi-head attention kernel for 8 Trainium2 NeuronCores.

Problem: x[8,1024,768] -> fused qkv proj -> 12-head attention (n=1024,
d_head=64) -> out proj + bias. Data-parallel over batch: core b handles x[b].

All device layouts are chosen so no on-device transposes are needed:
  - host supplies x transposed (xT [768,1024])
  - phase 1a computes [q;k]^T [1536,1024] (w stationary, xT moving)
  - phase 1b computes v_aug [1024, 12*65] (xT stationary, w_v moving),
    with a ones-column per head (rank-1 K=1 matmul) so the PV matmul
    also yields softmax row-sums
  - dots produce S^T[j,i] per head; K=64 head pairs are row-packed into
    the 128-row PE array; exp runs on ScalarE with the 1/sqrt(d) scale
    folded into the activation's free affine
  - PV uses v_aug as stationary (M=65): rows 0..63 = unnormalized out^T,
    row 64 = row-sum r; normalize via reciprocal + K=1 broadcast matmul
    + vector multiply
  - out projection uses A^T tiles directly as lhsT; bias via K=1 ones
    matmul against b_out
"""
import numpy as np

import concourse.bass as bass
import concourse.mybir as mybir
import concourse.tile as tile
from concourse import bacc

# The activation-table insertion pass picks sets greedily, which thrashes
# between exp_and_others and natural_log (~2.7us per reload) when a kernel
# interleaves Exp and Ln. Restrict Exp/Ln coverage to the combined set so
# one load serves the whole kernel. Set ids are positional, so entries are
# edited in place, never removed.
if not hasattr(bacc, "_orig_get_act_tables"):
    bacc._orig_get_act_tables = bacc.get_activation_tables


def _combined_act_tables(arch):
    tables = dict(bacc._orig_get_act_tables(arch))
    exp = mybir.ActivationFunctionType.Exp
    ln = mybir.ActivationFunctionType.Ln
    both = tables.get("natural_log_exp_and_others")
    if not both or exp not in both or ln not in both:
        return tables  # combined set unavailable: keep default behavior
    for name, funcs in tables.items():
        if name != "natural_log_exp_and_others" and (exp in funcs or ln in funcs):
            tables[name] = funcs - {exp, ln}
    return tables


bacc.get_activation_tables = _combined_act_tables

DIM = 768
HEADS = 12
DH = 64
INNER = HEADS * DH
N_TOK = 1024
BATCH = 8
N_CORES = 8
SCALE = DH ** -0.5
VW = HEADS * (DH + 1)  # 780: v columns with per-head ones column

F32 = mybir.dt.float32
MM_DT = mybir.dt.float16  # matmul operand dtype (2-byte: 1024-col moving ops)


_NSPLIT_CAP = 512


def _nsplits(total, cap=None):
    if cap is None:
        cap = _NSPLIT_CAP
    out = []
    s = 0
    while s < total:
        e = min(s + cap, total)
        out.append((s, e))
        s = e
    return out


def build_nc(n_tok=N_TOK, num_devices=N_CORES, mm_dt=MM_DT, debug=False,
             phases=(1, 2, 3), pairs=HEADS // 2,
             qkp_bufs=6, exps_bufs=9, psa_bufs=2, pv_bufs=2, lookahead=2,
             reps=1, usb_copy=True, fused=True, ablag=6, skip_norm=False,
             qk_copy_act=True, bcast_gpsimd=False, bias_dve=True,
             fused_ab=True, lagi=6, fill_even=True, norm_split=False,
             p3_deep=True, cap=512, norm_pool=1):
    global _NSPLIT_CAP
    _NSPLIT_CAP = cap
    nc = bacc.Bacc("TRN2", target_bir_lowering=False, debug=debug,
                   num_devices=num_devices)
    nt = n_tok // 128          # token tiles (i and j)
    KT = DIM // 128            # contraction tiles over model dim
    ET = (2 * DIM) // 128      # q+k rows -> 12 tiles
    ATT = INNER // 128         # A^T e-tiles -> 6

    # inputs are declared in the matmul dtype so the plain DMA load
    # satisfies the "rounded to FP32r" producer rule
    xT_d = nc.dram_tensor("xT", (DIM, n_tok), mm_dt, kind="ExternalInput")
    wqk_d = nc.dram_tensor("wqk", (DIM, 2 * DIM), mm_dt, kind="ExternalInput")
    wv_d = nc.dram_tensor("wv", (DIM, VW), mm_dt, kind="ExternalInput")
    ones_d = nc.dram_tensor("ones", (1, 128), mm_dt, kind="ExternalInput")
    wout_d = nc.dram_tensor("wout", (INNER, DIM), mm_dt, kind="ExternalInput")
    bout_d = nc.dram_tensor("bout", (1, DIM), mm_dt, kind="ExternalInput")
    o_d = nc.dram_tensor("o", (n_tok, DIM), F32, kind="ExternalOutput")

    use2 = 2 in phases
    npairs = pairs if use2 else 0

    with tile.TileContext(nc) as tc:
        with (
            tc.tile_pool(name="persist", bufs=1) as pp,
            tc.tile_pool(name="qkp", bufs=qkp_bufs) as qkp,
            tc.tile_pool(name="exps", bufs=exps_bufs) as exps,
            tc.tile_pool(name="p2s", bufs=1) as p2s,
            tc.tile_pool(name="psa", bufs=psa_bufs, space="PSUM") as psa,
            tc.tile_pool(name="pspv", bufs=pv_bufs, space="PSUM") as pspv,
        ):
            v_aug = pp.tile([128, nt, VW], mm_dt)    # v + ones cols, by i-tile
            a_T = pp.tile([128, ATT, n_tok], mm_dt)  # normalized attn out ^T
            smalls = pp.tile([1, 128 + DIM], mm_dt)
            ones = smalls[:, 0:128]
            bout = smalls[:, 128:]
            wout_sb = pp.tile([128, ATT, DIM], mm_dt)
            import contextlib as _ctl
            _loop = (tc.For_i(0, reps, 1,
                              hint_engines=(mybir.EngineType.PE,
                                            mybir.EngineType.Activation,
                                            mybir.EngineType.DVE))
                     if reps > 1 else _ctl.nullcontext())
            with _loop:
                nc.sync.dma_start(ones, ones_d.ap())
                nc.sync.dma_start(bout, bout_d.ap())

                qk_tiles = {}

                with tc.tile_pool(name="p1", bufs=1) as p1:
                    # ---------------- phase 1: qkv projections ----------------
                    if 1 in phases:
                        xt = p1.tile([128, KT, n_tok], mm_dt)
                        wv = p1.tile([128, KT, VW], mm_dt)
                        wqk_all = p1.tile([128, KT, 2 * DIM], mm_dt,
                                          name="wqkall")
                        # one DMA per 128-row block: HWDGE descriptor issue
                        # (~625ns each, one shared device) paces phase 1, so
                        # fewer/bigger descriptors beat fine-grained chunks
                        if fused:
                            for kt in range(KT):
                                nc.sync.dma_start(
                                    xt[:, kt, :],
                                    xT_d.ap()[kt * 128:(kt + 1) * 128, :])
                                nc.sync.dma_start(
                                    wqk_all[:, kt, :],
                                    wqk_d.ap()[kt * 128:(kt + 1) * 128, :])
                            for kt in range(KT):
                                nc.sync.dma_start(
                                    wv[:, kt, :],
                                    wv_d.ap()[kt * 128:(kt + 1) * 128, :])
                            # wout rides at the end of the weight stream so
                            # p3 never waits on DMA
                            for kt in range(ATT):
                                nc.sync.dma_start(
                                    wout_sb[:, kt, :],
                                    wout_d.ap()[kt * 128:(kt + 1) * 128, :])
                        else:
                            # sequential driver runs 1b first: stream xt+wv
                            # so 1b chains pipeline with the DMA, then qk
                            for kt in range(KT):
                                nc.sync.dma_start(
                                    xt[:, kt, :],
                                    xT_d.ap()[kt * 128:(kt + 1) * 128, :])
                                nc.sync.dma_start(
                                    wv[:, kt, :],
                                    wv_d.ap()[kt * 128:(kt + 1) * 128, :])
                            for kt in range(KT):
                                nc.sync.dma_start(
                                    wqk_all[:, kt, :],
                                    wqk_d.ap()[kt * 128:(kt + 1) * 128, :])

                    def emit_1b_one(it):
                        # v_aug[i, c] = sum_d xT[d, i] * wv[d, c].  The
                        # per-head ones columns (softmax row-sum trick) are
                        # zero in wv, so the PSUM copy writes 0 there; a tiny
                        # strided memset then sets them to 1.0 — cheaper than
                        # the old K=1 vind matmul on the binding PE.
                        pv_ = psa.tile([128, 1024], F32, tag="psa")
                        for kt in range(KT):
                            for (s, e) in _nsplits(VW):
                                nc.tensor.matmul(
                                    pv_[:, s:e],
                                    xt[:, kt, it * 128:(it + 1) * 128],
                                    wv[:, kt, s:e],
                                    start=(kt == 0), stop=(kt == KT - 1),
                                )
                        if qk_copy_act:
                            nc.scalar.copy(v_aug[:, it, :], pv_[:, 0:VW])
                        else:
                            nc.vector.tensor_copy(v_aug[:, it, :],
                                                  pv_[:, 0:VW])
                        nc.vector.memset(v_aug[:, it, DH::DH + 1], 1.0)

                    def emit_1b():
                        for it in range(nt):
                            emit_1b_one(it)

                    def emit_1a_one(mt):
                        # qkT[e, i] = sum_d wqk[d, e] * xT[d, i], one 128-row tile
                        if True:
                            pq = psa.tile([128, 1024], F32, tag="psa")
                            wo = ((mt % (ET // 2)) * 256
                                  + (128 if mt >= ET // 2 else 0))
                            for kt in range(KT):
                                for (s, e) in _nsplits(n_tok):
                                    nc.tensor.matmul(
                                        pq[:, s:e],
                                        wqk_all[:, kt, wo:wo + 128],
                                        xt[:, kt, s:e],
                                        start=(kt == 0), stop=(kt == KT - 1),
                                    )
                            qt = qkp.tile([128, n_tok], mm_dt, tag="qk",
                                          name=f"qk{mt}")
                            if qk_copy_act:
                                # PSUM->SBUF on ScalarE: faster PSUM port,
                                # and keeps the copy off the DVE queue where
                                # it stalls the next pair's dots
                                nc.scalar.copy(qt[:], pq[:, 0:n_tok])
                            else:
                                nc.vector.tensor_copy(qt[:], pq[:, 0:n_tok])
                            qk_tiles[mt] = qt

                    def emit_1a_pair(pr):
                        for mt in (pr, ET // 2 + pr):
                            emit_1a_one(mt)

                    # --- phase 2: one unit = one head. Global modulo software
                    # pipeline: every step emits one dots (PE+ACT producer), an
                    # optional filler (1b / 1a-prefetch chain), and pops one
                    # deferred consumer (PV matmul or normalization) from a
                    # global queue that trails LAG steps behind, so PE never
                    # waits on the exp of the tile it just produced and the
                    # work mix per step is uniform across unit boundaries.
                    LAG = min(ablag, nt)
                    workq = []

                    def qstep(force=False):
                        while workq and (force or len(workq) > LAG):
                            workq.pop(0)()

                    def emit_unit(pair, half, fillers):
                        qt = qk_tiles[pair]
                        kt_ = qk_tiles[ET // 2 + pair]
                        h = 2 * pair + half
                        p0 = half * 64
                        up = pspv.tile([65, 1024], F32, tag="pv", name=f"up{h}")
                        ets = {}

                        def dots(jt):
                            ps = psa.tile([128, 1024], F32, tag="psa")
                            for (s, e) in _nsplits(n_tok):
                                nc.tensor.matmul(
                                    ps[:, s:e],
                                    kt_[p0:p0 + 64, jt * 128:(jt + 1) * 128],
                                    qt[p0:p0 + 64, s:e],
                                    start=True, stop=True,
                                )
                            et = exps.tile([128, n_tok], mm_dt, tag="expS",
                                           name=f"et{half}_{jt}")
                            nc.scalar.activation(
                                et[:], ps[:, 0:n_tok],
                                mybir.ActivationFunctionType.Exp, scale=SCALE)
                            ets[jt] = et

                        def pv(jt):
                            for (s, e) in _nsplits(n_tok):
                                nc.tensor.matmul(
                                    up[:, s:e],
                                    v_aug[:, jt, h * 65:h * 65 + 65],
                                    ets[jt][:, s:e],
                                    start=(jt == 0), stop=(jt == nt - 1),
                                )
                            del ets[jt]

                        def norm():
                            # a_T[h rows] = up[0:64] * (1 / up[64]).
                            # 1/r via exp(-ln r) on ScalarE: a [1, n] DVE
                            # reciprocal runs on one lane at 8 cyc/elem
                            # (~8.5 us) and would serialize the pipeline.
                            lnr = p2s.tile([1, n_tok], F32, tag="lnr")
                            nc.scalar.activation(
                                lnr[:], up[64:65, 0:n_tok],
                                mybir.ActivationFunctionType.Ln)
                            rinv = p2s.tile([1, n_tok], mm_dt, tag="rinv")
                            nc.scalar.activation(
                                rinv[:], lnr[:],
                                mybir.ActivationFunctionType.Exp, scale=-1.0)
                            if usb_copy:
                                usb = p2s.tile([64, n_tok], F32, tag="usb")
                                nc.vector.tensor_copy(usb[:], up[0:64, 0:n_tok])
                                mul_in = usb[:]
                            else:
                                mul_in = up[0:64, 0:n_tok]
                            bc = psa.tile([128, 1024], F32, tag="psa")
                            for (s, e) in _nsplits(n_tok):
                                nc.tensor.matmul(
                                    bc[0:64, s:e], ones[0:1, 0:64],
                                    rinv[0:1, s:e], start=True, stop=True,
                                )
                            nc.vector.tensor_mul(
                                a_T[p0:p0 + 64, h // 2, :], mul_in,
                                bc[0:64, 0:n_tok])

                        for jt in range(nt):
                            dots(jt)
                            if fillers:
                                fillers.pop(0)()
                            workq.append(lambda jt=jt: pv(jt))
                            qstep()
                        while fillers:
                            fillers.pop(0)()
                        workq.append(norm)

                    def emit_pair_ab(pair):
                        # both heads of a pair interleaved per j-tile:
                        # alternating PE row groups overlap fill/drain, and
                        # exp feeds stay dense. PSUM: 2 dots tiles + 2 up
                        # accumulators = 8 banks.
                        qt = qk_tiles[pair]
                        kt_ = qk_tiles[ET // 2 + pair]
                        ups = {}
                        etsd = {0: {}, 1: {}}
                        for half in (0, 1):
                            ups[half] = pspv.tile([65, 1024], F32, tag="pv",
                                                  name=f"upab{half}")

                        def dots(half, jt):
                            p0 = half * 64
                            ps = psa.tile([128, 1024], F32, tag="psa")
                            for (s, e) in _nsplits(n_tok):
                                nc.tensor.matmul(
                                    ps[:, s:e],
                                    kt_[p0:p0 + 64, jt * 128:(jt + 1) * 128],
                                    qt[p0:p0 + 64, s:e],
                                    start=True, stop=True,
                                )
                            et = exps.tile([128, n_tok], mm_dt, tag="expS",
                                           name=f"etab{half}_{jt}")
                            nc.scalar.activation(
                                et[:], ps[:, 0:n_tok],
                                mybir.ActivationFunctionType.Exp, scale=SCALE)
                            etsd[half][jt] = et

                        def pv(half, jt):
                            h = 2 * pair + half
                            for (s, e) in _nsplits(n_tok):
                                nc.tensor.matmul(
                                    ups[half][:, s:e],
                                    v_aug[:, jt, h * 65:h * 65 + 65],
                                    etsd[half][jt][:, s:e],
                                    start=(jt == 0), stop=(jt == nt - 1),
                                )
                            del etsd[half][jt]

                        def norm(half):
                            h = 2 * pair + half
                            p0 = half * 64
                            up = ups[half]
                            lnr = p2s.tile([1, n_tok], F32, tag="lnr")
                            nc.scalar.activation(
                                lnr[:], up[64:65, 0:n_tok],
                                mybir.ActivationFunctionType.Ln)
                            rinv = p2s.tile([1, n_tok], mm_dt, tag="rinv")
                            nc.scalar.activation(
                                rinv[:], lnr[:],
                                mybir.ActivationFunctionType.Exp, scale=-1.0)
                            usb = p2s.tile([64, n_tok], F32, tag="usb")
                            nc.vector.tensor_copy(usb[:], up[0:64, 0:n_tok])
                            dst = a_T[p0:p0 + 64, h // 2, :]
                            if bcast_gpsimd:
                                # broadcast 1/r on the otherwise-idle GPSIMD,
                                # then multiply in place on DVE
                                nc.gpsimd.partition_broadcast(
                                    dst.bitcast(F32), rinv.bitcast(F32)[:])
                                nc.vector.tensor_mul(dst, dst.bitcast(F32),
                                                     usb[:])
                            else:
                                bc = psa.tile([128, 1024], F32, tag="psa")
                                for (s, e) in _nsplits(n_tok):
                                    nc.tensor.matmul(
                                        bc[0:64, s:e], ones[0:1, 0:64],
                                        rinv[0:1, s:e], start=True, stop=True,
                                    )
                                nc.vector.tensor_mul(
                                    dst, usb[:], bc[0:64, 0:n_tok])

                        ABLAG = ablag
                        for jt in range(nt):
                            dots(0, jt)
                            dots(1, jt)
                            if jt >= ABLAG:
                                pv(0, jt - ABLAG)
                                pv(1, jt - ABLAG)
                        for jt in range(max(nt - ABLAG, 0), nt):
                            pv(0, jt)
                            pv(1, jt)
                        if not skip_norm:
                            norm(0)
                            norm(1)

                    def emit_pair_fused(pair, fillers, lagi=6):
                        # both halves interleaved per j-tile (row-group
                        # alternation overlaps PE fill/drain) while keeping
                        # the fused filler structure. PSUM: 2 dots tiles +
                        # 2 up accumulators = 8 banks.
                        qt = qk_tiles[pair]
                        kt_ = qk_tiles[ET // 2 + pair]
                        ups = {}
                        for half in (0, 1):
                            ups[half] = pspv.tile([65, 1024], F32, tag="pv",
                                                  name=f"upf{half}")
                        ets = {}

                        def dots(half, jt):
                            p0 = half * 64
                            ps = psa.tile([128, 1024], F32, tag="psa")
                            for (s, e) in _nsplits(n_tok):
                                nc.tensor.matmul(
                                    ps[:, s:e],
                                    kt_[p0:p0 + 64, jt * 128:(jt + 1) * 128],
                                    qt[p0:p0 + 64, s:e],
                                    start=True, stop=True,
                                )
                            et = exps.tile([128, n_tok], mm_dt, tag="expS",
                                           name=f"etf{half}_{jt}")
                            nc.scalar.activation(
                                et[:], ps[:, 0:n_tok],
                                mybir.ActivationFunctionType.Exp, scale=SCALE)
                            ets[(half, jt)] = et

                        def pv(half, jt):
                            h = 2 * pair + half
                            for (s, e) in _nsplits(n_tok):
                                nc.tensor.matmul(
                                    ups[half][:, s:e],
                                    v_aug[:, jt, h * 65:h * 65 + 65],
                                    ets[(half, jt)][:, s:e],
                                    start=(jt == 0), stop=(jt == nt - 1),
                                )
                            del ets[(half, jt)]

                        rinvs = {}
                        usbs = {}

                        def norm_act(half):
                            up = ups[half]
                            if norm_pool:
                                # copy out^T + r row to SBUF (frees the PSUM
                                # bank in one op), 1/r approx on DVE: no ACT
                                # table pressure, no [1,n] ln/exp lane waste
                                usb = p2s.tile([65, n_tok], F32,
                                               tag=f"usb{half}",
                                               name=f"usbf{half}")
                                nc.vector.tensor_copy(usb[:],
                                                      up[0:65, 0:n_tok])
                                rinv = p2s.tile([1, n_tok], F32,
                                                tag=f"rinv{half}",
                                                name=f"rinvf{half}")
                                if norm_pool == 3:
                                    lnr = p2s.tile([1, n_tok], F32,
                                                   tag=f"lnr{half}")
                                    nc.scalar.activation(
                                        lnr[:], usb[64:65, :],
                                        mybir.ActivationFunctionType.Ln)
                                    nc.scalar.activation(
                                        rinv[:], lnr[:],
                                        mybir.ActivationFunctionType.Exp,
                                        scale=-1.0)
                                else:
                                    # custom DVE / gpsimd ISA ops read the
                                    # memloc's partition 0 regardless of the
                                    # AP offset: stage the r row (partition
                                    # 64) to a partition-0 tile via an
                                    # SBUF->SBUF DMA (off-engine, idle in
                                    # phase 2) before the DVE reciprocal
                                    rrow = p2s.tile([1, n_tok], F32,
                                                    tag=f"rrow{half}",
                                                    name=f"rrowf{half}")
                                    nc.sync.dma_start(rrow[:],
                                                      usb[64:65, :])
                                    nc.vector.reciprocal_approx_fast(
                                        rinv[:], rrow[:])
                                usbs[half] = usb
                                rinvs[half] = rinv
                                return
                            lnr = p2s.tile([1, n_tok], F32, tag="lnr")
                            nc.scalar.activation(
                                lnr[:], up[64:65, 0:n_tok],
                                mybir.ActivationFunctionType.Ln)
                            rinv = p2s.tile([1, n_tok], mm_dt,
                                            tag=(f"rinv{half}" if norm_split
                                                 else "rinv"),
                                            name=f"rinvf{half}")
                            nc.scalar.activation(
                                rinv[:], lnr[:],
                                mybir.ActivationFunctionType.Exp, scale=-1.0)
                            rinvs[half] = rinv

                        def norm_rest(half):
                            h = 2 * pair + half
                            p0 = half * 64
                            up = ups[half]
                            rinv = rinvs[half]
                            if norm_pool:
                                # broadcast 1/r on the idle Pool/GPSIMD,
                                # multiply on DVE
                                usb = usbs[half]
                                if norm_pool == 2:
                                    rin16 = p2s.tile([1, n_tok], mm_dt,
                                                     tag=f"ri16{half}")
                                    nc.vector.tensor_copy(rin16[:], rinv[:])
                                    bcp = psa.tile([128, 1024], F32,
                                                   tag="psa")
                                    for (s, e) in _nsplits(n_tok):
                                        nc.tensor.matmul(
                                            bcp[0:64, s:e], ones[0:1, 0:64],
                                            rin16[0:1, s:e],
                                            start=True, stop=True)
                                    nc.vector.tensor_mul(
                                        a_T[p0:p0 + 64, h // 2, :],
                                        usb[0:64, :], bcp[0:64, 0:n_tok])
                                    return
                                bc = p2s.tile([64, n_tok], F32,
                                              tag=f"bc{half}",
                                              name=f"bcf{half}")
                                nc.gpsimd.partition_broadcast(bc[:], rinv[:])
                                nc.vector.tensor_mul(
                                    a_T[p0:p0 + 64, h // 2, :],
                                    usb[0:64, :], bc[:])
                                return
                            usb = p2s.tile([64, n_tok], F32, tag="usb")
                            nc.vector.tensor_copy(usb[:], up[0:64, 0:n_tok])
                            bc = psa.tile([128, 1024], F32, tag="psa")
                            for (s, e) in _nsplits(n_tok):
                                nc.tensor.matmul(
                                    bc[0:64, s:e], ones[0:1, 0:64],
                                    rinv[0:1, s:e], start=True, stop=True,
                                )
                            nc.vector.tensor_mul(
                                a_T[p0:p0 + 64, h // 2, :], usb[:],
                                bc[0:64, 0:n_tok])

                        for jt in range(nt):
                            dots(0, jt)
                            dots(1, jt)
                            if fillers:
                                fillers.pop(0)()
                            workq.append(lambda jt=jt: pv(0, jt))
                            workq.append(lambda jt=jt: pv(1, jt))
                            while len(workq) > lagi:
                                workq.pop(0)()
                        while fillers:
                            fillers.pop(0)()
                        if norm_split:
                            workq.append(lambda: norm_act(0))
                            workq.append(lambda: norm_act(1))
                            workq.append(lambda: norm_rest(0))
                            workq.append(lambda: norm_rest(1))
                        else:
                            workq.append(lambda: (norm_act(0), norm_rest(0)))
                            workq.append(lambda: (norm_act(1), norm_rest(1)))

                    # software-pipelined emission driver
                    if 1 in phases and use2:
                        emit_1a_pair(0)
                        emit_1a_pair(1)
                        # filler queues: 1b chains ride inside pair 0; 1a
                        # prefetch for pair pr rides inside pair pr-2, half B
                        fill = {}
                        for pair in range(npairs):
                            for half in (0, 1):
                                fill[(pair, half)] = []
                        for it in range(nt):
                            u = (0, 0) if it < 6 else (0, 1)
                            fill[u].append(lambda it=it: emit_1b_one(it))
                        for pr in range(2, ET // 2):
                            host = (pr - 2, 1)
                            if host not in fill:
                                host = (npairs - 1, 1)
                            fill[host].append(lambda m=pr: emit_1a_one(m))
                            fill[host].append(
                                lambda m=ET // 2 + pr: emit_1a_one(m))
                        if fused_ab:
                            if fill_even:
                                # re-spread: 1b 0..5 in pair 0; 1b 6,7 +
                                # 1a(2) in pair 1; 1a(pr) in pair pr-2
                                fl = {p: [] for p in range(npairs)}
                                fl[0] = [lambda it=it: emit_1b_one(it)
                                         for it in range(min(6, nt))]
                                if npairs > 1:
                                    fl[1] = ([lambda it=it: emit_1b_one(it)
                                              for it in range(6, nt)] +
                                             [lambda: emit_1a_one(2),
                                              lambda: emit_1a_one(ET // 2 + 2)])
                                for pr in range(3, ET // 2):
                                    host = min(pr - 2, npairs - 1)
                                    fl[host].append(
                                        lambda m=pr: emit_1a_one(m))
                                    fl[host].append(
                                        lambda m=ET // 2 + pr: emit_1a_one(m))
                                for pair in range(npairs):
                                    emit_pair_fused(pair, fl[pair], lagi=lagi)
                            else:
                                for pair in range(npairs):
                                    emit_pair_fused(
                                        pair,
                                        fill[(pair, 0)] + fill[(pair, 1)],
                                        lagi=lagi)
                        else:
                            for pair in range(npairs):
                                for half in (0, 1):
                                    emit_unit(pair, half, fill[(pair, half)])
                        qstep(force=True)
                    elif 1 in phases:
                        emit_1b()
                        for pr in range(ET // 2):
                            emit_1a_pair(pr)
                    else:
                        for pair in range(npairs):
                            for half in (0, 1):
                                emit_unit(pair, half, [])
                        qstep(force=True)

                # ------------- phase 3: output projection + bias -------------
                with (
                    tc.tile_pool(name="pw", bufs=1) as pw,
                    tc.tile_pool(name="p3o", bufs=3) as p3o,
                ):
                    if 3 in phases:
                        wout = wout_sb
                        if 1 not in phases:
                            for kt in range(ATT):
                                nc.sync.dma_start(
                                    wout[:, kt, :],
                                    wout_d.ap()[kt * 128:(kt + 1) * 128, :])
                        if bias_dve:
                            # bias broadcast built once; the i-tile loop adds
                            # it on the otherwise-idle DVE instead of 16 K=1
                            # matmuls on the (binding) PE
                            bias_bc = pw.tile([128, DIM], F32)
                            bps = psa.tile([128, 1024], F32, tag="psa")
                            for (s, e) in _nsplits(DIM):
                                nc.tensor.matmul(
                                    bps[0:128, s:e], ones[0:1, 0:128],
                                    bout[0:1, s:e], start=True, stop=True,
                                )
                            nc.scalar.copy(bias_bc[:], bps[:, 0:DIM])
                        for it in range(nt):
                            if p3_deep and it % 2 == 1:
                                # the PV accumulator banks are idle in p3:
                                # alternate output accumulators across both
                                # pools for a deeper pipeline
                                po = pspv.tile([128, 1024], F32, tag="pv",
                                               name="po_b")
                            else:
                                po = psa.tile([128, 1024], F32, tag="psa")
                            for kt in range(ATT):
                                for (s, e) in _nsplits(DIM):
                                    nc.tensor.matmul(
                                        po[:, s:e],
                                        a_T[:, kt, it * 128:(it + 1) * 128],
                                        wout[:, kt, s:e],
                                        start=(kt == 0),
                                        stop=(bias_dve and kt == ATT - 1),
                                    )
                            if not bias_dve:
                                for (s, e) in _nsplits(DIM):
                                    nc.tensor.matmul(
                                        po[:, s:e], ones[0:1, 0:128],
                                        bout[0:1, s:e],
                                        start=False, stop=True,
                                    )
                            osb = p3o.tile([128, DIM], F32, tag="osb")
                            if bias_dve:
                                nc.vector.tensor_add(osb[:], po[:, 0:DIM],
                                                     bias_bc[:])
                            else:
                                nc.scalar.copy(osb[:], po[:, 0:DIM])
                            nc.sync.dma_start(
                                o_d.ap()[it * 128:(it + 1) * 128, :], osb[:])


    nc.compile()
    return nc


def host_prep(x, w_qkv, w_out, b_out, batch=BATCH):
    """Build per-core input maps from the full problem inputs."""
    x = np.asarray(x, dtype=np.float32)
    w_qkv = np.asarray(w_qkv, dtype=np.float32)
    w_out = np.asarray(w_out, dtype=np.float32)
    b_out = np.asarray(b_out, dtype=np.float32)

    w_q = w_qkv[:, 0:INNER]
    w_k = w_qkv[:, INNER:2 * INNER]
    w_v = w_qkv[:, 2 * INNER:3 * INNER]
    wqk = np.zeros((DIM, 2 * INNER), dtype=np.float32)
    for p in range(HEADS // 2):
        wqk[:, p * 256:p * 256 + 128] = w_q[:, p * 128:(p + 1) * 128]
        wqk[:, p * 256 + 128:(p + 1) * 256] = w_k[:, p * 128:(p + 1) * 128]
    wv = np.zeros((DIM, VW), dtype=np.float32)
    for h in range(HEADS):
        wv[:, h * 65:h * 65 + 64] = w_v[:, h * 64:(h + 1) * 64]
    hd = np.float16  # matmul operand dtype on device (must match MM_DT)
    shared = {
        "ones": np.ones((1, 128), dtype=hd),
        "wqk": wqk.astype(hd),
        "wv": wv.astype(hd),
        "wout": np.ascontiguousarray(w_out).astype(hd),
        "bout": np.ascontiguousarray(b_out.reshape(1, DIM)).astype(hd),
    }
    in_maps = []
    for b in range(batch):
        m = dict(shared)
        m["xT"] = np.ascontiguousarray(x[b].T).astype(hd)
        in_maps.append(m)
    return in_maps


# --- inline PJRT runner (build once, call many) ---
def _make_runner(nc, n_cores):
    import jax
    from jax.sharding import Mesh, PartitionSpec
    from jax.experimental.shard_map import shard_map
    from concourse import bass2jax

    bass2jax.install_neuronx_cc_hook()
    partition_name = nc.partition_id_tensor.name if nc.partition_id_tensor else None
    in_names, out_names, out_avals, zero_outs = [], [], [], []
    for alloc in nc.m.functions[0].allocations:
        if not isinstance(alloc, mybir.MemoryLocationSet):
            continue
        name = alloc.memorylocations[0].name
        if alloc.kind == "ExternalInput":
            if name != partition_name:
                in_names.append(name)
        elif alloc.kind == "ExternalOutput":
            shape = tuple(alloc.tensor_shape)
            dtype = mybir.dt.np(alloc.dtype)
            out_names.append(name)
            out_avals.append(jax.core.ShapedArray(shape, dtype))
            zero_outs.append(np.zeros(shape, dtype))
    n_params = len(in_names)
    n_outs = len(out_avals)
    all_in_names = list(in_names) + list(out_names)
    if partition_name is not None:
        all_in_names.append(partition_name)

    def _body(*args):
        operands = list(args)
        if partition_name is not None:
            operands.append(bass2jax.partition_id_tensor())
        outs = bass2jax._bass_exec_p.bind(
            *operands,
            out_avals=tuple(out_avals),
            in_names=tuple(all_in_names),
            out_names=tuple(out_names),
            lowering_input_output_aliases=(),
            sim_require_finite=True,
            sim_require_nnan=True,
            nc=nc,
        )
        return tuple(outs)

    donate = tuple(range(n_params, n_params + n_outs))
    if n_cores == 1:
        fn = jax.jit(_body, donate_argnums=donate, keep_unused=True)

        def run(in_maps):
            args = [np.asarray(in_maps[0][n]) for n in in_names]
            out_arrs = fn(*args, *[z.copy() for z in zero_outs])
            jax.block_until_ready(out_arrs)
            return [{n: np.asarray(out_arrs[i]) for i, n in enumerate(out_names)}]
        return run

    devices = jax.devices()[:n_cores]
    mesh = Mesh(np.asarray(devices), ("core",))
    in_specs = (PartitionSpec("core"),) * (n_params + n_outs)
    out_specs = (PartitionSpec("core"),) * n_outs
    fn = jax.jit(
        shard_map(_body, mesh=mesh, in_specs=in_specs, out_specs=out_specs,
                  check_rep=False),
        donate_argnums=donate, keep_unused=True,
    )

    def run(in_maps):
        per_core = [[np.asarray(m[n]) for n in in_names] for m in in_maps]
        concat_in = [
            np.concatenate([per_core[c][i] for c in range(n_cores)], axis=0)
            for i in range(n_params)
        ]
        concat_zeros = [
            np.zeros((n_cores * z.shape[0], *z.shape[1:]), z.dtype)
            for z in zero_outs
        ]
        out_arrs = fn(*concat_in, *concat_zeros)
        jax.block_until_ready(out_arrs)
        return [
            {n: np.asarray(out_arrs[i]).reshape(n_cores, *out_avals[i].shape)[c]
             for i, n in enumerate(out_names)}
            for c in range(n_cores)
        ]
    return run


_CACHE = {}


def get_runner():
    if "run" not in _CACHE:
        nc = build_nc()
        _CACHE["nc"] = nc
        _CACHE["run"] = _make_runner(nc, N_CORES)
    return _CACHE["run"]


def kernel(x, w_qkv, w_out, b_out):
    run = get_runner()
    in_maps = host_prep(x, w_qkv, w_out, b_out)
    res = run(in_maps)
    return np.stack([res[b]["o"] for b in range(BATCH)], axis=0)



# revision 22
# speedup vs baseline: 1.2460x; 1.0010x over previous
"""Self-contained multi-head attention kernel for 8 Trainium2 NeuronCores.

Problem: x[8,1024,768] -> fused qkv proj -> 12-head attention (n=1024,
d_head=64) -> out proj + bias. Data-parallel over batch: core b handles x[b].

All device layouts are chosen so no on-device transposes are needed:
  - host supplies x transposed (xT [768,1024])
  - phase 1a computes [q;k]^T [1536,1024] (w stationary, xT moving)
  - phase 1b computes v_aug [1024, 12*65] (xT stationary, w_v moving),
    with a ones-column per head (rank-1 K=1 matmul) so the PV matmul
    also yields softmax row-sums
  - dots produce S^T[j,i] per head; K=64 head pairs are row-packed into
    the 128-row PE array; exp runs on ScalarE with the 1/sqrt(d) scale
    folded into the activation's free affine
  - PV uses v_aug as stationary (M=65): rows 0..63 = unnormalized out^T,
    row 64 = row-sum r; normalize via reciprocal + K=1 broadcast matmul
    + vector multiply
  - out projection uses A^T tiles directly as lhsT; bias via K=1 ones
    matmul against b_out
"""
import numpy as np

import concourse.bass as bass
import concourse.mybir as mybir
import concourse.tile as tile
from concourse import bacc

# The activation-table insertion pass picks sets greedily, which thrashes
# between exp_and_others and natural_log (~2.7us per reload) when a kernel
# interleaves Exp and Ln. Restrict Exp/Ln coverage to the combined set so
# one load serves the whole kernel. Set ids are positional, so entries are
# edited in place, never removed.
if not hasattr(bacc, "_orig_get_act_tables"):
    bacc._orig_get_act_tables = bacc.get_activation_tables


def _combined_act_tables(arch):
    tables = dict(bacc._orig_get_act_tables(arch))
    exp = mybir.ActivationFunctionType.Exp
    ln = mybir.ActivationFunctionType.Ln
    both = tables.get("natural_log_exp_and_others")
    if not both or exp not in both or ln not in both:
        return tables  # combined set unavailable: keep default behavior
    for name, funcs in tables.items():
        if name != "natural_log_exp_and_others" and (exp in funcs or ln in funcs):
            tables[name] = funcs - {exp, ln}
    return tables


bacc.get_activation_tables = _combined_act_tables

DIM = 768
HEADS = 12
DH = 64
INNER = HEADS * DH
N_TOK = 1024
BATCH = 8
N_CORES = 8
SCALE = DH ** -0.5
VW = HEADS * (DH + 1)  # 780: v columns with per-head ones column

F32 = mybir.dt.float32
MM_DT = mybir.dt.float16  # matmul operand dtype (2-byte: 1024-col moving ops)


_NSPLIT_CAP = 512


def _nsplits(total, cap=None):
    if cap is None:
        cap = _NSPLIT_CAP
    out = []
    s = 0
    while s < total:
        e = min(s + cap, total)
        out.append((s, e))
        s = e
    return out


def build_nc(n_tok=N_TOK, num_devices=N_CORES, mm_dt=MM_DT, debug=False,
             phases=(1, 2, 3), pairs=HEADS // 2,
             qkp_bufs=6, exps_bufs=9, psa_bufs=2, pv_bufs=2, lookahead=2,
             reps=1, usb_copy=True, fused=True, ablag=6, skip_norm=False,
             qk_copy_act=True, bcast_gpsimd=False, bias_dve=True,
             fused_ab=True, lagi=6, fill_even=True, norm_split=False,
             p3_deep=True, cap=512, norm_pool=1):
    global _NSPLIT_CAP
    _NSPLIT_CAP = cap
    nc = bacc.Bacc("TRN2", target_bir_lowering=False, debug=debug,
                   num_devices=num_devices)
    nt = n_tok // 128          # token tiles (i and j)
    KT = DIM // 128            # contraction tiles over model dim
    ET = (2 * DIM) // 128      # q+k rows -> 12 tiles
    ATT = INNER // 128         # A^T e-tiles -> 6

    # inputs are declared in the matmul dtype so the plain DMA load
    # satisfies the "rounded to FP32r" producer rule
    xT_d = nc.dram_tensor("xT", (DIM, n_tok), mm_dt, kind="ExternalInput")
    wqk_d = nc.dram_tensor("wqk", (DIM, 2 * DIM), mm_dt, kind="ExternalInput")
    wv_d = nc.dram_tensor("wv", (DIM, VW), mm_dt, kind="ExternalInput")
    ones_d = nc.dram_tensor("ones", (1, 128), mm_dt, kind="ExternalInput")
    wout_d = nc.dram_tensor("wout", (INNER, DIM), mm_dt, kind="ExternalInput")
    bout_d = nc.dram_tensor("bout", (1, DIM), mm_dt, kind="ExternalInput")
    o_d = nc.dram_tensor("o", (n_tok, DIM), mm_dt, kind="ExternalOutput")

    use2 = 2 in phases
    npairs = pairs if use2 else 0

    with tile.TileContext(nc) as tc:
        with (
            tc.tile_pool(name="persist", bufs=1) as pp,
            tc.tile_pool(name="qkp", bufs=qkp_bufs) as qkp,
            tc.tile_pool(name="exps", bufs=exps_bufs) as exps,
            tc.tile_pool(name="p2s", bufs=1) as p2s,
            tc.tile_pool(name="psa", bufs=psa_bufs, space="PSUM") as psa,
            tc.tile_pool(name="pspv", bufs=pv_bufs, space="PSUM") as pspv,
        ):
            v_aug = pp.tile([128, nt, VW], mm_dt)    # v + ones cols, by i-tile
            a_T = pp.tile([128, ATT, n_tok], mm_dt)  # normalized attn out ^T
            smalls = pp.tile([1, 128 + DIM], mm_dt)
            ones = smalls[:, 0:128]
            bout = smalls[:, 128:]
            wout_sb = pp.tile([128, ATT, DIM], mm_dt)
            import contextlib as _ctl
            _loop = (tc.For_i(0, reps, 1,
                              hint_engines=(mybir.EngineType.PE,
                                            mybir.EngineType.Activation,
                                            mybir.EngineType.DVE))
                     if reps > 1 else _ctl.nullcontext())
            with _loop:
                nc.sync.dma_start(ones, ones_d.ap())
                nc.sync.dma_start(bout, bout_d.ap())

                qk_tiles = {}

                with tc.tile_pool(name="p1", bufs=1) as p1:
                    # ---------------- phase 1: qkv projections ----------------
                    if 1 in phases:
                        xt = p1.tile([128, KT, n_tok], mm_dt)
                        wv = p1.tile([128, KT, VW], mm_dt)
                        wqk_all = p1.tile([128, KT, 2 * DIM], mm_dt,
                                          name="wqkall")
                        # one DMA per 128-row block: HWDGE descriptor issue
                        # (~625ns each, one shared device) paces phase 1, so
                        # fewer/bigger descriptors beat fine-grained chunks
                        if fused:
                            for kt in range(KT):
                                nc.sync.dma_start(
                                    xt[:, kt, :],
                                    xT_d.ap()[kt * 128:(kt + 1) * 128, :])
                                nc.sync.dma_start(
                                    wqk_all[:, kt, :],
                                    wqk_d.ap()[kt * 128:(kt + 1) * 128, :])
                            for kt in range(KT):
                                nc.sync.dma_start(
                                    wv[:, kt, :],
                                    wv_d.ap()[kt * 128:(kt + 1) * 128, :])
                            # wout rides at the end of the weight stream so
                            # p3 never waits on DMA
                            for kt in range(ATT):
                                nc.sync.dma_start(
                                    wout_sb[:, kt, :],
                                    wout_d.ap()[kt * 128:(kt + 1) * 128, :])
                        else:
                            # sequential driver runs 1b first: stream xt+wv
                            # so 1b chains pipeline with the DMA, then qk
                            for kt in range(KT):
                                nc.sync.dma_start(
                                    xt[:, kt, :],
                                    xT_d.ap()[kt * 128:(kt + 1) * 128, :])
                                nc.sync.dma_start(
                                    wv[:, kt, :],
                                    wv_d.ap()[kt * 128:(kt + 1) * 128, :])
                            for kt in range(KT):
                                nc.sync.dma_start(
                                    wqk_all[:, kt, :],
                                    wqk_d.ap()[kt * 128:(kt + 1) * 128, :])

                    def emit_1b_one(it):
                        # v_aug[i, c] = sum_d xT[d, i] * wv[d, c].  The
                        # per-head ones columns (softmax row-sum trick) are
                        # zero in wv, so the PSUM copy writes 0 there; a tiny
                        # strided memset then sets them to 1.0 — cheaper than
                        # the old K=1 vind matmul on the binding PE.
                        pv_ = psa.tile([128, 1024], F32, tag="psa")
                        for kt in range(KT):
                            for (s, e) in _nsplits(VW):
                                nc.tensor.matmul(
                                    pv_[:, s:e],
                                    xt[:, kt, it * 128:(it + 1) * 128],
                                    wv[:, kt, s:e],
                                    start=(kt == 0), stop=(kt == KT - 1),
                                )
                        if qk_copy_act:
                            nc.scalar.copy(v_aug[:, it, :], pv_[:, 0:VW])
                        else:
                            nc.vector.tensor_copy(v_aug[:, it, :],
                                                  pv_[:, 0:VW])
                        nc.vector.memset(v_aug[:, it, DH::DH + 1], 1.0)

                    def emit_1b():
                        for it in range(nt):
                            emit_1b_one(it)

                    def emit_1a_one(mt):
                        # qkT[e, i] = sum_d wqk[d, e] * xT[d, i], one 128-row tile
                        if True:
                            pq = psa.tile([128, 1024], F32, tag="psa")
                            wo = ((mt % (ET // 2)) * 256
                                  + (128 if mt >= ET // 2 else 0))
                            for kt in range(KT):
                                for (s, e) in _nsplits(n_tok):
                                    nc.tensor.matmul(
                                        pq[:, s:e],
                                        wqk_all[:, kt, wo:wo + 128],
                                        xt[:, kt, s:e],
                                        start=(kt == 0), stop=(kt == KT - 1),
                                    )
                            qt = qkp.tile([128, n_tok], mm_dt, tag="qk",
                                          name=f"qk{mt}")
                            if qk_copy_act:
                                # PSUM->SBUF on ScalarE: faster PSUM port,
                                # and keeps the copy off the DVE queue where
                                # it stalls the next pair's dots
                                nc.scalar.copy(qt[:], pq[:, 0:n_tok])
                            else:
                                nc.vector.tensor_copy(qt[:], pq[:, 0:n_tok])
                            qk_tiles[mt] = qt

                    def emit_1a_pair(pr):
                        for mt in (pr, ET // 2 + pr):
                            emit_1a_one(mt)

                    # --- phase 2: one unit = one head. Global modulo software
                    # pipeline: every step emits one dots (PE+ACT producer), an
                    # optional filler (1b / 1a-prefetch chain), and pops one
                    # deferred consumer (PV matmul or normalization) from a
                    # global queue that trails LAG steps behind, so PE never
                    # waits on the exp of the tile it just produced and the
                    # work mix per step is uniform across unit boundaries.
                    LAG = min(ablag, nt)
                    workq = []

                    def qstep(force=False):
                        while workq and (force or len(workq) > LAG):
                            workq.pop(0)()

                    def emit_unit(pair, half, fillers):
                        qt = qk_tiles[pair]
                        kt_ = qk_tiles[ET // 2 + pair]
                        h = 2 * pair + half
                        p0 = half * 64
                        up = pspv.tile([65, 1024], F32, tag="pv", name=f"up{h}")
                        ets = {}

                        def dots(jt):
                            ps = psa.tile([128, 1024], F32, tag="psa")
                            for (s, e) in _nsplits(n_tok):
                                nc.tensor.matmul(
                                    ps[:, s:e],
                                    kt_[p0:p0 + 64, jt * 128:(jt + 1) * 128],
                                    qt[p0:p0 + 64, s:e],
                                    start=True, stop=True,
                                )
                            et = exps.tile([128, n_tok], mm_dt, tag="expS",
                                           name=f"et{half}_{jt}")
                            nc.scalar.activation(
                                et[:], ps[:, 0:n_tok],
                                mybir.ActivationFunctionType.Exp, scale=SCALE)
                            ets[jt] = et

                        def pv(jt):
                            for (s, e) in _nsplits(n_tok):
                                nc.tensor.matmul(
                                    up[:, s:e],
                                    v_aug[:, jt, h * 65:h * 65 + 65],
                                    ets[jt][:, s:e],
                                    start=(jt == 0), stop=(jt == nt - 1),
                                )
                            del ets[jt]

                        def norm():
                            # a_T[h rows] = up[0:64] * (1 / up[64]).
                            # 1/r via exp(-ln r) on ScalarE: a [1, n] DVE
                            # reciprocal runs on one lane at 8 cyc/elem
                            # (~8.5 us) and would serialize the pipeline.
                            lnr = p2s.tile([1, n_tok], F32, tag="lnr")
                            nc.scalar.activation(
                                lnr[:], up[64:65, 0:n_tok],
                                mybir.ActivationFunctionType.Ln)
                            rinv = p2s.tile([1, n_tok], mm_dt, tag="rinv")
                            nc.scalar.activation(
                                rinv[:], lnr[:],
                                mybir.ActivationFunctionType.Exp, scale=-1.0)
                            if usb_copy:
                                usb = p2s.tile([64, n_tok], F32, tag="usb")
                                nc.vector.tensor_copy(usb[:], up[0:64, 0:n_tok])
                                mul_in = usb[:]
                            else:
                                mul_in = up[0:64, 0:n_tok]
                            bc = psa.tile([128, 1024], F32, tag="psa")
                            for (s, e) in _nsplits(n_tok):
                                nc.tensor.matmul(
                                    bc[0:64, s:e], ones[0:1, 0:64],
                                    rinv[0:1, s:e], start=True, stop=True,
                                )
                            nc.vector.tensor_mul(
                                a_T[p0:p0 + 64, h // 2, :], mul_in,
                                bc[0:64, 0:n_tok])

                        for jt in range(nt):
                            dots(jt)
                            if fillers:
                                fillers.pop(0)()
                            workq.append(lambda jt=jt: pv(jt))
                            qstep()
                        while fillers:
                            fillers.pop(0)()
                        workq.append(norm)

                    def emit_pair_ab(pair):
                        # both heads of a pair interleaved per j-tile:
                        # alternating PE row groups overlap fill/drain, and
                        # exp feeds stay dense. PSUM: 2 dots tiles + 2 up
                        # accumulators = 8 banks.
                        qt = qk_tiles[pair]
                        kt_ = qk_tiles[ET // 2 + pair]
                        ups = {}
                        etsd = {0: {}, 1: {}}
                        for half in (0, 1):
                            ups[half] = pspv.tile([65, 1024], F32, tag="pv",
                                                  name=f"upab{half}")

                        def dots(half, jt):
                            p0 = half * 64
                            ps = psa.tile([128, 1024], F32, tag="psa")
                            for (s, e) in _nsplits(n_tok):
                                nc.tensor.matmul(
                                    ps[:, s:e],
                                    kt_[p0:p0 + 64, jt * 128:(jt + 1) * 128],
                                    qt[p0:p0 + 64, s:e],
                                    start=True, stop=True,
                                )
                            et = exps.tile([128, n_tok], mm_dt, tag="expS",
                                           name=f"etab{half}_{jt}")
                            nc.scalar.activation(
                                et[:], ps[:, 0:n_tok],
                                mybir.ActivationFunctionType.Exp, scale=SCALE)
                            etsd[half][jt] = et

                        def pv(half, jt):
                            h = 2 * pair + half
                            for (s, e) in _nsplits(n_tok):
                                nc.tensor.matmul(
                                    ups[half][:, s:e],
                                    v_aug[:, jt, h * 65:h * 65 + 65],
                                    etsd[half][jt][:, s:e],
                                    start=(jt == 0), stop=(jt == nt - 1),
                                )
                            del etsd[half][jt]

                        def norm(half):
                            h = 2 * pair + half
                            p0 = half * 64
                            up = ups[half]
                            lnr = p2s.tile([1, n_tok], F32, tag="lnr")
                            nc.scalar.activation(
                                lnr[:], up[64:65, 0:n_tok],
                                mybir.ActivationFunctionType.Ln)
                            rinv = p2s.tile([1, n_tok], mm_dt, tag="rinv")
                            nc.scalar.activation(
                                rinv[:], lnr[:],
                                mybir.ActivationFunctionType.Exp, scale=-1.0)
                            usb = p2s.tile([64, n_tok], F32, tag="usb")
                            nc.vector.tensor_copy(usb[:], up[0:64, 0:n_tok])
                            dst = a_T[p0:p0 + 64, h // 2, :]
                            if bcast_gpsimd:
                                # broadcast 1/r on the otherwise-idle GPSIMD,
                                # then multiply in place on DVE
                                nc.gpsimd.partition_broadcast(
                                    dst.bitcast(F32), rinv.bitcast(F32)[:])
                                nc.vector.tensor_mul(dst, dst.bitcast(F32),
                                                     usb[:])
                            else:
                                bc = psa.tile([128, 1024], F32, tag="psa")
                                for (s, e) in _nsplits(n_tok):
                                    nc.tensor.matmul(
                                        bc[0:64, s:e], ones[0:1, 0:64],
                                        rinv[0:1, s:e], start=True, stop=True,
                                    )
                                nc.vector.tensor_mul(
                                    dst, usb[:], bc[0:64, 0:n_tok])

                        ABLAG = ablag
                        for jt in range(nt):
                            dots(0, jt)
                            dots(1, jt)
                            if jt >= ABLAG:
                                pv(0, jt - ABLAG)
                                pv(1, jt - ABLAG)
                        for jt in range(max(nt - ABLAG, 0), nt):
                            pv(0, jt)
                            pv(1, jt)
                        if not skip_norm:
                            norm(0)
                            norm(1)

                    def emit_pair_fused(pair, fillers, lagi=6):
                        # both halves interleaved per j-tile (row-group
                        # alternation overlaps PE fill/drain) while keeping
                        # the fused filler structure. PSUM: 2 dots tiles +
                        # 2 up accumulators = 8 banks.
                        qt = qk_tiles[pair]
                        kt_ = qk_tiles[ET // 2 + pair]
                        ups = {}
                        for half in (0, 1):
                            ups[half] = pspv.tile([65, 1024], F32, tag="pv",
                                                  name=f"upf{half}")
                        ets = {}

                        def dots(half, jt):
                            p0 = half * 64
                            ps = psa.tile([128, 1024], F32, tag="psa")
                            for (s, e) in _nsplits(n_tok):
                                nc.tensor.matmul(
                                    ps[:, s:e],
                                    kt_[p0:p0 + 64, jt * 128:(jt + 1) * 128],
                                    qt[p0:p0 + 64, s:e],
                                    start=True, stop=True,
                                )
                            et = exps.tile([128, n_tok], mm_dt, tag="expS",
                                           name=f"etf{half}_{jt}")
                            nc.scalar.activation(
                                et[:], ps[:, 0:n_tok],
                                mybir.ActivationFunctionType.Exp, scale=SCALE)
                            ets[(half, jt)] = et

                        def pv(half, jt):
                            h = 2 * pair + half
                            for (s, e) in _nsplits(n_tok):
                                nc.tensor.matmul(
                                    ups[half][:, s:e],
                                    v_aug[:, jt, h * 65:h * 65 + 65],
                                    ets[(half, jt)][:, s:e],
                                    start=(jt == 0), stop=(jt == nt - 1),
                                )
                            del ets[(half, jt)]

                        rinvs = {}
                        usbs = {}

                        def norm_act(half):
                            up = ups[half]
                            if norm_pool:
                                # copy out^T + r row to SBUF (frees the PSUM
                                # bank in one op), 1/r approx on DVE: no ACT
                                # table pressure, no [1,n] ln/exp lane waste
                                usb = p2s.tile([65, n_tok], F32,
                                               tag=f"usb{half}",
                                               name=f"usbf{half}")
                                nc.vector.tensor_copy(usb[:],
                                                      up[0:65, 0:n_tok])
                                rinv = p2s.tile([1, n_tok], F32,
                                                tag=f"rinv{half}",
                                                name=f"rinvf{half}")
                                if norm_pool == 3:
                                    lnr = p2s.tile([1, n_tok], F32,
                                                   tag=f"lnr{half}")
                                    nc.scalar.activation(
                                        lnr[:], usb[64:65, :],
                                        mybir.ActivationFunctionType.Ln)
                                    nc.scalar.activation(
                                        rinv[:], lnr[:],
                                        mybir.ActivationFunctionType.Exp,
                                        scale=-1.0)
                                else:
                                    # custom DVE / gpsimd ISA ops read the
                                    # memloc's partition 0 regardless of the
                                    # AP offset: stage the r row (partition
                                    # 64) to a partition-0 tile via an
                                    # SBUF->SBUF DMA (off-engine, idle in
                                    # phase 2) before the DVE reciprocal
                                    rrow = p2s.tile([1, n_tok], F32,
                                                    tag=f"rrow{half}",
                                                    name=f"rrowf{half}")
                                    nc.sync.dma_start(rrow[:],
                                                      usb[64:65, :])
                                    nc.vector.reciprocal_approx_fast(
                                        rinv[:], rrow[:])
                                usbs[half] = usb
                                rinvs[half] = rinv
                                return
                            lnr = p2s.tile([1, n_tok], F32, tag="lnr")
                            nc.scalar.activation(
                                lnr[:], up[64:65, 0:n_tok],
                                mybir.ActivationFunctionType.Ln)
                            rinv = p2s.tile([1, n_tok], mm_dt,
                                            tag=(f"rinv{half}" if norm_split
                                                 else "rinv"),
                                            name=f"rinvf{half}")
                            nc.scalar.activation(
                                rinv[:], lnr[:],
                                mybir.ActivationFunctionType.Exp, scale=-1.0)
                            rinvs[half] = rinv

                        def norm_rest(half):
                            h = 2 * pair + half
                            p0 = half * 64
                            up = ups[half]
                            rinv = rinvs[half]
                            if norm_pool:
                                # broadcast 1/r on the idle Pool/GPSIMD,
                                # multiply on DVE
                                usb = usbs[half]
                                if norm_pool == 2:
                                    rin16 = p2s.tile([1, n_tok], mm_dt,
                                                     tag=f"ri16{half}")
                                    nc.vector.tensor_copy(rin16[:], rinv[:])
                                    bcp = psa.tile([128, 1024], F32,
                                                   tag="psa")
                                    for (s, e) in _nsplits(n_tok):
                                        nc.tensor.matmul(
                                            bcp[0:64, s:e], ones[0:1, 0:64],
                                            rin16[0:1, s:e],
                                            start=True, stop=True)
                                    nc.vector.tensor_mul(
                                        a_T[p0:p0 + 64, h // 2, :],
                                        usb[0:64, :], bcp[0:64, 0:n_tok])
                                    return
                                bc = p2s.tile([64, n_tok], F32,
                                              tag=f"bc{half}",
                                              name=f"bcf{half}")
                                nc.gpsimd.partition_broadcast(bc[:], rinv[:])
                                nc.vector.tensor_mul(
                                    a_T[p0:p0 + 64, h // 2, :],
                                    usb[0:64, :], bc[:])
                                return
                            usb = p2s.tile([64, n_tok], F32, tag="usb")
                            nc.vector.tensor_copy(usb[:], up[0:64, 0:n_tok])
                            bc = psa.tile([128, 1024], F32, tag="psa")
                            for (s, e) in _nsplits(n_tok):
                                nc.tensor.matmul(
                                    bc[0:64, s:e], ones[0:1, 0:64],
                                    rinv[0:1, s:e], start=True, stop=True,
                                )
                            nc.vector.tensor_mul(
                                a_T[p0:p0 + 64, h // 2, :], usb[:],
                                bc[0:64, 0:n_tok])

                        for jt in range(nt):
                            dots(0, jt)
                            dots(1, jt)
                            if fillers:
                                fillers.pop(0)()
                            workq.append(lambda jt=jt: pv(0, jt))
                            workq.append(lambda jt=jt: pv(1, jt))
                            while len(workq) > lagi:
                                workq.pop(0)()
                        while fillers:
                            fillers.pop(0)()
                        if norm_split:
                            workq.append(lambda: norm_act(0))
                            workq.append(lambda: norm_act(1))
                            workq.append(lambda: norm_rest(0))
                            workq.append(lambda: norm_rest(1))
                        else:
                            workq.append(lambda: (norm_act(0), norm_rest(0)))
                            workq.append(lambda: (norm_act(1), norm_rest(1)))

                    # software-pipelined emission driver
                    if 1 in phases and use2:
                        emit_1a_pair(0)
                        emit_1a_pair(1)
                        # filler queues: 1b chains ride inside pair 0; 1a
                        # prefetch for pair pr rides inside pair pr-2, half B
                        fill = {}
                        for pair in range(npairs):
                            for half in (0, 1):
                                fill[(pair, half)] = []
                        for it in range(nt):
                            u = (0, 0) if it < 6 else (0, 1)
                            fill[u].append(lambda it=it: emit_1b_one(it))
                        for pr in range(2, ET // 2):
                            host = (pr - 2, 1)
                            if host not in fill:
                                host = (npairs - 1, 1)
                            fill[host].append(lambda m=pr: emit_1a_one(m))
                            fill[host].append(
                                lambda m=ET // 2 + pr: emit_1a_one(m))
                        if fused_ab:
                            if fill_even:
                                # re-spread: 1b 0..5 in pair 0; 1b 6,7 +
                                # 1a(2) in pair 1; 1a(pr) in pair pr-2
                                fl = {p: [] for p in range(npairs)}
                                fl[0] = [lambda it=it: emit_1b_one(it)
                                         for it in range(min(6, nt))]
                                if npairs > 1:
                                    fl[1] = ([lambda it=it: emit_1b_one(it)
                                              for it in range(6, nt)] +
                                             [lambda: emit_1a_one(2),
                                              lambda: emit_1a_one(ET // 2 + 2)])
                                for pr in range(3, ET // 2):
                                    host = min(pr - 2, npairs - 1)
                                    fl[host].append(
                                        lambda m=pr: emit_1a_one(m))
                                    fl[host].append(
                                        lambda m=ET // 2 + pr: emit_1a_one(m))
                                for pair in range(npairs):
                                    emit_pair_fused(pair, fl[pair], lagi=lagi)
                            else:
                                for pair in range(npairs):
                                    emit_pair_fused(
                                        pair,
                                        fill[(pair, 0)] + fill[(pair, 1)],
                                        lagi=lagi)
                        else:
                            for pair in range(npairs):
                                for half in (0, 1):
                                    emit_unit(pair, half, fill[(pair, half)])
                        qstep(force=True)
                    elif 1 in phases:
                        emit_1b()
                        for pr in range(ET // 2):
                            emit_1a_pair(pr)
                    else:
                        for pair in range(npairs):
                            for half in (0, 1):
                                emit_unit(pair, half, [])
                        qstep(force=True)

                # ------------- phase 3: output projection + bias -------------
                with (
                    tc.tile_pool(name="pw", bufs=1) as pw,
                    tc.tile_pool(name="p3o", bufs=3) as p3o,
                ):
                    if 3 in phases:
                        wout = wout_sb
                        if 1 not in phases:
                            for kt in range(ATT):
                                nc.sync.dma_start(
                                    wout[:, kt, :],
                                    wout_d.ap()[kt * 128:(kt + 1) * 128, :])
                        if bias_dve:
                            # bias broadcast built once; the i-tile loop adds
                            # it on the otherwise-idle DVE instead of 16 K=1
                            # matmuls on the (binding) PE
                            bias_bc = pw.tile([128, DIM], F32)
                            bps = psa.tile([128, 1024], F32, tag="psa")
                            for (s, e) in _nsplits(DIM):
                                nc.tensor.matmul(
                                    bps[0:128, s:e], ones[0:1, 0:128],
                                    bout[0:1, s:e], start=True, stop=True,
                                )
                            nc.scalar.copy(bias_bc[:], bps[:, 0:DIM])
                        for it in range(nt):
                            if p3_deep and it % 2 == 1:
                                # the PV accumulator banks are idle in p3:
                                # alternate output accumulators across both
                                # pools for a deeper pipeline
                                po = pspv.tile([128, 1024], F32, tag="pv",
                                               name="po_b")
                            else:
                                po = psa.tile([128, 1024], F32, tag="psa")
                            for kt in range(ATT):
                                for (s, e) in _nsplits(DIM):
                                    nc.tensor.matmul(
                                        po[:, s:e],
                                        a_T[:, kt, it * 128:(it + 1) * 128],
                                        wout[:, kt, s:e],
                                        start=(kt == 0),
                                        stop=(bias_dve and kt == ATT - 1),
                                    )
                            if not bias_dve:
                                for (s, e) in _nsplits(DIM):
                                    nc.tensor.matmul(
                                        po[:, s:e], ones[0:1, 0:128],
                                        bout[0:1, s:e],
                                        start=False, stop=True,
                                    )
                            osb = p3o.tile([128, DIM], mm_dt, tag="osb")
                            if bias_dve:
                                nc.vector.tensor_add(osb[:], po[:, 0:DIM],
                                                     bias_bc[:])
                            else:
                                nc.scalar.copy(osb[:], po[:, 0:DIM])
                            nc.sync.dma_start(
                                o_d.ap()[it * 128:(it + 1) * 128, :], osb[:])


    nc.compile()
    return nc


def host_prep(x, w_qkv, w_out, b_out, batch=BATCH):
    """Build per-core input maps from the full problem inputs."""
    x = np.asarray(x, dtype=np.float32)
    w_qkv = np.asarray(w_qkv, dtype=np.float32)
    w_out = np.asarray(w_out, dtype=np.float32)
    b_out = np.asarray(b_out, dtype=np.float32)

    w_q = w_qkv[:, 0:INNER]
    w_k = w_qkv[:, INNER:2 * INNER]
    w_v = w_qkv[:, 2 * INNER:3 * INNER]
    wqk = np.zeros((DIM, 2 * INNER), dtype=np.float32)
    for p in range(HEADS // 2):
        wqk[:, p * 256:p * 256 + 128] = w_q[:, p * 128:(p + 1) * 128]
        wqk[:, p * 256 + 128:(p + 1) * 256] = w_k[:, p * 128:(p + 1) * 128]
    wv = np.zeros((DIM, VW), dtype=np.float32)
    for h in range(HEADS):
        wv[:, h * 65:h * 65 + 64] = w_v[:, h * 64:(h + 1) * 64]
    hd = np.float16  # matmul operand dtype on device (must match MM_DT)
    shared = {
        "ones": np.ones((1, 128), dtype=hd),
        "wqk": wqk.astype(hd),
        "wv": wv.astype(hd),
        "wout": np.ascontiguousarray(w_out).astype(hd),
        "bout": np.ascontiguousarray(b_out.reshape(1, DIM)).astype(hd),
    }
    in_maps = []
    for b in range(batch):
        m = dict(shared)
        m["xT"] = np.ascontiguousarray(x[b].T).astype(hd)
        in_maps.append(m)
    return in_maps


# --- inline PJRT runner (build once, call many) ---
def _make_runner(nc, n_cores):
    import jax
    from jax.sharding import Mesh, PartitionSpec
    from jax.experimental.shard_map import shard_map
    from concourse import bass2jax

    bass2jax.install_neuronx_cc_hook()
    partition_name = nc.partition_id_tensor.name if nc.partition_id_tensor else None
    in_names, out_names, out_avals, zero_outs = [], [], [], []
    for alloc in nc.m.functions[0].allocations:
        if not isinstance(alloc, mybir.MemoryLocationSet):
            continue
        name = alloc.memorylocations[0].name
        if alloc.kind == "ExternalInput":
            if name != partition_name:
                in_names.append(name)
        elif alloc.kind == "ExternalOutput":
            shape = tuple(alloc.tensor_shape)
            dtype = mybir.dt.np(alloc.dtype)
            out_names.append(name)
            out_avals.append(jax.core.ShapedArray(shape, dtype))
            zero_outs.append(np.zeros(shape, dtype))
    n_params = len(in_names)
    n_outs = len(out_avals)
    all_in_names = list(in_names) + list(out_names)
    if partition_name is not None:
        all_in_names.append(partition_name)

    def _body(*args):
        operands = list(args)
        if partition_name is not None:
            operands.append(bass2jax.partition_id_tensor())
        outs = bass2jax._bass_exec_p.bind(
            *operands,
            out_avals=tuple(out_avals),
            in_names=tuple(all_in_names),
            out_names=tuple(out_names),
            lowering_input_output_aliases=(),
            sim_require_finite=True,
            sim_require_nnan=True,
            nc=nc,
        )
        return tuple(outs)

    donate = tuple(range(n_params, n_params + n_outs))
    if n_cores == 1:
        fn = jax.jit(_body, donate_argnums=donate, keep_unused=True)

        def run(in_maps):
            args = [np.asarray(in_maps[0][n]) for n in in_names]
            out_arrs = fn(*args, *[z.copy() for z in zero_outs])
            jax.block_until_ready(out_arrs)
            return [{n: np.asarray(out_arrs[i]) for i, n in enumerate(out_names)}]
        return run

    devices = jax.devices()[:n_cores]
    mesh = Mesh(np.asarray(devices), ("core",))
    in_specs = (PartitionSpec("core"),) * (n_params + n_outs)
    out_specs = (PartitionSpec("core"),) * n_outs
    fn = jax.jit(
        shard_map(_body, mesh=mesh, in_specs=in_specs, out_specs=out_specs,
                  check_rep=False),
        donate_argnums=donate, keep_unused=True,
    )

    def run(in_maps):
        per_core = [[np.asarray(m[n]) for n in in_names] for m in in_maps]
        concat_in = [
            np.concatenate([per_core[c][i] for c in range(n_cores)], axis=0)
            for i in range(n_params)
        ]
        concat_zeros = [
            np.zeros((n_cores * z.shape[0], *z.shape[1:]), z.dtype)
            for z in zero_outs
        ]
        out_arrs = fn(*concat_in, *concat_zeros)
        jax.block_until_ready(out_arrs)
        return [
            {n: np.asarray(out_arrs[i]).reshape(n_cores, *out_avals[i].shape)[c]
             for i, n in enumerate(out_names)}
            for c in range(n_cores)
        ]
    return run


_CACHE = {}


def get_runner():
    if "run" not in _CACHE:
        nc = build_nc()
        _CACHE["nc"] = nc
        _CACHE["run"] = _make_runner(nc, N_CORES)
    return _CACHE["run"]


def kernel(x, w_qkv, w_out, b_out):
    run = get_runner()
    in_maps = host_prep(x, w_qkv, w_out, b_out)
    res = run(in_maps)
    return np.stack([res[b]["o"] for b in range(BATCH)],
                    axis=0).astype(np.float32)



# revision 24
# speedup vs baseline: 1.2487x; 1.0021x over previous
"""Self-contained multi-head attention kernel for 8 Trainium2 NeuronCores.

Problem: x[8,1024,768] -> fused qkv proj -> 12-head attention (n=1024,
d_head=64) -> out proj + bias. Data-parallel over batch: core b handles x[b].

All device layouts are chosen so no on-device transposes are needed:
  - host supplies x transposed (xT [768,1024])
  - phase 1a computes [q;k]^T [1536,1024] (w stationary, xT moving)
  - phase 1b computes v_aug [1024, 12*65] (xT stationary, w_v moving),
    with a ones-column per head (rank-1 K=1 matmul) so the PV matmul
    also yields softmax row-sums
  - dots produce S^T[j,i] per head; K=64 head pairs are row-packed into
    the 128-row PE array; exp runs on ScalarE with the 1/sqrt(d) scale
    folded into the activation's free affine
  - PV uses v_aug as stationary (M=65): rows 0..63 = unnormalized out^T,
    row 64 = row-sum r; normalize via reciprocal + K=1 broadcast matmul
    + vector multiply
  - out projection uses A^T tiles directly as lhsT; bias via K=1 ones
    matmul against b_out
"""
import numpy as np

import concourse.bass as bass
import concourse.mybir as mybir
import concourse.tile as tile
from concourse import bacc

# The activation-table insertion pass picks sets greedily, which thrashes
# between exp_and_others and natural_log (~2.7us per reload) when a kernel
# interleaves Exp and Ln. Restrict Exp/Ln coverage to the combined set so
# one load serves the whole kernel. Set ids are positional, so entries are
# edited in place, never removed.
if not hasattr(bacc, "_orig_get_act_tables"):
    bacc._orig_get_act_tables = bacc.get_activation_tables


def _combined_act_tables(arch):
    tables = dict(bacc._orig_get_act_tables(arch))
    exp = mybir.ActivationFunctionType.Exp
    ln = mybir.ActivationFunctionType.Ln
    both = tables.get("natural_log_exp_and_others")
    if not both or exp not in both or ln not in both:
        return tables  # combined set unavailable: keep default behavior
    for name, funcs in tables.items():
        if name != "natural_log_exp_and_others" and (exp in funcs or ln in funcs):
            tables[name] = funcs - {exp, ln}
    return tables


bacc.get_activation_tables = _combined_act_tables

DIM = 768
HEADS = 12
DH = 64
INNER = HEADS * DH
N_TOK = 1024
BATCH = 8
N_CORES = 8
SCALE = DH ** -0.5
VW = HEADS * (DH + 1)  # 780: v columns with per-head ones column

F32 = mybir.dt.float32
MM_DT = mybir.dt.float16  # matmul operand dtype (2-byte: 1024-col moving ops)


_NSPLIT_CAP = 512


def _nsplits(total, cap=None):
    if cap is None:
        cap = _NSPLIT_CAP
    out = []
    s = 0
    while s < total:
        e = min(s + cap, total)
        out.append((s, e))
        s = e
    return out


def build_nc(n_tok=N_TOK, num_devices=N_CORES, mm_dt=MM_DT, debug=False,
             phases=(1, 2, 3), pairs=HEADS // 2,
             qkp_bufs=8, exps_bufs=12, psa_bufs=2, pv_bufs=2, lookahead=2,
             reps=1, usb_copy=True, fused=True, ablag=6, skip_norm=False,
             qk_copy_act=True, bcast_gpsimd=False, bias_dve=True,
             fused_ab=True, lagi=8, fill_even=True, norm_split=False,
             p3_deep=True, cap=512, norm_pool=1):
    global _NSPLIT_CAP
    _NSPLIT_CAP = cap
    nc = bacc.Bacc("TRN2", target_bir_lowering=False, debug=debug,
                   num_devices=num_devices)
    nt = n_tok // 128          # token tiles (i and j)
    KT = DIM // 128            # contraction tiles over model dim
    ET = (2 * DIM) // 128      # q+k rows -> 12 tiles
    ATT = INNER // 128         # A^T e-tiles -> 6

    # inputs are declared in the matmul dtype so the plain DMA load
    # satisfies the "rounded to FP32r" producer rule
    xT_d = nc.dram_tensor("xT", (DIM, n_tok), mm_dt, kind="ExternalInput")
    wqk_d = nc.dram_tensor("wqk", (DIM, 2 * DIM), mm_dt, kind="ExternalInput")
    wv_d = nc.dram_tensor("wv", (DIM, VW), mm_dt, kind="ExternalInput")
    ones_d = nc.dram_tensor("ones", (1, 128), mm_dt, kind="ExternalInput")
    wout_d = nc.dram_tensor("wout", (INNER, DIM), mm_dt, kind="ExternalInput")
    bout_d = nc.dram_tensor("bout", (1, DIM), mm_dt, kind="ExternalInput")
    o_d = nc.dram_tensor("o", (n_tok, DIM), mm_dt, kind="ExternalOutput")

    use2 = 2 in phases
    npairs = pairs if use2 else 0

    with tile.TileContext(nc) as tc:
        with (
            tc.tile_pool(name="persist", bufs=1) as pp,
            tc.tile_pool(name="qkp", bufs=qkp_bufs) as qkp,
            tc.tile_pool(name="exps", bufs=exps_bufs) as exps,
            tc.tile_pool(name="p2s", bufs=1) as p2s,
            tc.tile_pool(name="psa", bufs=psa_bufs, space="PSUM") as psa,
            tc.tile_pool(name="pspv", bufs=pv_bufs, space="PSUM") as pspv,
        ):
            v_aug = pp.tile([128, nt, VW], mm_dt)    # v + ones cols, by i-tile
            a_T = pp.tile([128, ATT, n_tok], mm_dt)  # normalized attn out ^T
            smalls = pp.tile([1, 128 + DIM], mm_dt)
            ones = smalls[:, 0:128]
            bout = smalls[:, 128:]
            wout_sb = pp.tile([128, ATT, DIM], mm_dt)
            import contextlib as _ctl
            _loop = (tc.For_i(0, reps, 1,
                              hint_engines=(mybir.EngineType.PE,
                                            mybir.EngineType.Activation,
                                            mybir.EngineType.DVE))
                     if reps > 1 else _ctl.nullcontext())
            with _loop:
                nc.sync.dma_start(ones, ones_d.ap())
                nc.sync.dma_start(bout, bout_d.ap())

                qk_tiles = {}

                with tc.tile_pool(name="p1", bufs=1) as p1:
                    # ---------------- phase 1: qkv projections ----------------
                    if 1 in phases:
                        xt = p1.tile([128, KT, n_tok], mm_dt)
                        wv = p1.tile([128, KT, VW], mm_dt)
                        wqk_all = p1.tile([128, KT, 2 * DIM], mm_dt,
                                          name="wqkall")
                        # one DMA per 128-row block: HWDGE descriptor issue
                        # (~625ns each, one shared device) paces phase 1, so
                        # fewer/bigger descriptors beat fine-grained chunks
                        if fused:
                            for kt in range(KT):
                                nc.sync.dma_start(
                                    xt[:, kt, :],
                                    xT_d.ap()[kt * 128:(kt + 1) * 128, :])
                                nc.sync.dma_start(
                                    wqk_all[:, kt, :],
                                    wqk_d.ap()[kt * 128:(kt + 1) * 128, :])
                            for kt in range(KT):
                                nc.sync.dma_start(
                                    wv[:, kt, :],
                                    wv_d.ap()[kt * 128:(kt + 1) * 128, :])
                            # wout rides at the end of the weight stream so
                            # p3 never waits on DMA
                            for kt in range(ATT):
                                nc.sync.dma_start(
                                    wout_sb[:, kt, :],
                                    wout_d.ap()[kt * 128:(kt + 1) * 128, :])
                        else:
                            # sequential driver runs 1b first: stream xt+wv
                            # so 1b chains pipeline with the DMA, then qk
                            for kt in range(KT):
                                nc.sync.dma_start(
                                    xt[:, kt, :],
                                    xT_d.ap()[kt * 128:(kt + 1) * 128, :])
                                nc.sync.dma_start(
                                    wv[:, kt, :],
                                    wv_d.ap()[kt * 128:(kt + 1) * 128, :])
                            for kt in range(KT):
                                nc.sync.dma_start(
                                    wqk_all[:, kt, :],
                                    wqk_d.ap()[kt * 128:(kt + 1) * 128, :])

                    def emit_1b_one(it):
                        # v_aug[i, c] = sum_d xT[d, i] * wv[d, c].  The
                        # per-head ones columns (softmax row-sum trick) are
                        # zero in wv, so the PSUM copy writes 0 there; a tiny
                        # strided memset then sets them to 1.0 — cheaper than
                        # the old K=1 vind matmul on the binding PE.
                        pv_ = psa.tile([128, 1024], F32, tag="psa")
                        for kt in range(KT):
                            for (s, e) in _nsplits(VW):
                                nc.tensor.matmul(
                                    pv_[:, s:e],
                                    xt[:, kt, it * 128:(it + 1) * 128],
                                    wv[:, kt, s:e],
                                    start=(kt == 0), stop=(kt == KT - 1),
                                )
                        if qk_copy_act:
                            nc.scalar.copy(v_aug[:, it, :], pv_[:, 0:VW])
                        else:
                            nc.vector.tensor_copy(v_aug[:, it, :],
                                                  pv_[:, 0:VW])
                        nc.vector.memset(v_aug[:, it, DH::DH + 1], 1.0)

                    def emit_1b():
                        for it in range(nt):
                            emit_1b_one(it)

                    def emit_1a_one(mt):
                        # qkT[e, i] = sum_d wqk[d, e] * xT[d, i], one 128-row tile
                        if True:
                            pq = psa.tile([128, 1024], F32, tag="psa")
                            wo = ((mt % (ET // 2)) * 256
                                  + (128 if mt >= ET // 2 else 0))
                            for kt in range(KT):
                                for (s, e) in _nsplits(n_tok):
                                    nc.tensor.matmul(
                                        pq[:, s:e],
                                        wqk_all[:, kt, wo:wo + 128],
                                        xt[:, kt, s:e],
                                        start=(kt == 0), stop=(kt == KT - 1),
                                    )
                            qt = qkp.tile([128, n_tok], mm_dt, tag="qk",
                                          name=f"qk{mt}")
                            if qk_copy_act:
                                # PSUM->SBUF on ScalarE: faster PSUM port,
                                # and keeps the copy off the DVE queue where
                                # it stalls the next pair's dots
                                nc.scalar.copy(qt[:], pq[:, 0:n_tok])
                            else:
                                nc.vector.tensor_copy(qt[:], pq[:, 0:n_tok])
                            qk_tiles[mt] = qt

                    def emit_1a_pair(pr):
                        for mt in (pr, ET // 2 + pr):
                            emit_1a_one(mt)

                    # --- phase 2: one unit = one head. Global modulo software
                    # pipeline: every step emits one dots (PE+ACT producer), an
                    # optional filler (1b / 1a-prefetch chain), and pops one
                    # deferred consumer (PV matmul or normalization) from a
                    # global queue that trails LAG steps behind, so PE never
                    # waits on the exp of the tile it just produced and the
                    # work mix per step is uniform across unit boundaries.
                    LAG = min(ablag, nt)
                    workq = []

                    def qstep(force=False):
                        while workq and (force or len(workq) > LAG):
                            workq.pop(0)()

                    def emit_unit(pair, half, fillers):
                        qt = qk_tiles[pair]
                        kt_ = qk_tiles[ET // 2 + pair]
                        h = 2 * pair + half
                        p0 = half * 64
                        up = pspv.tile([65, 1024], F32, tag="pv", name=f"up{h}")
                        ets = {}

                        def dots(jt):
                            ps = psa.tile([128, 1024], F32, tag="psa")
                            for (s, e) in _nsplits(n_tok):
                                nc.tensor.matmul(
                                    ps[:, s:e],
                                    kt_[p0:p0 + 64, jt * 128:(jt + 1) * 128],
                                    qt[p0:p0 + 64, s:e],
                                    start=True, stop=True,
                                )
                            et = exps.tile([128, n_tok], mm_dt, tag="expS",
                                           name=f"et{half}_{jt}")
                            nc.scalar.activation(
                                et[:], ps[:, 0:n_tok],
                                mybir.ActivationFunctionType.Exp, scale=SCALE)
                            ets[jt] = et

                        def pv(jt):
                            for (s, e) in _nsplits(n_tok):
                                nc.tensor.matmul(
                                    up[:, s:e],
                                    v_aug[:, jt, h * 65:h * 65 + 65],
                                    ets[jt][:, s:e],
                                    start=(jt == 0), stop=(jt == nt - 1),
                                )
                            del ets[jt]

                        def norm():
                            # a_T[h rows] = up[0:64] * (1 / up[64]).
                            # 1/r via exp(-ln r) on ScalarE: a [1, n] DVE
                            # reciprocal runs on one lane at 8 cyc/elem
                            # (~8.5 us) and would serialize the pipeline.
                            lnr = p2s.tile([1, n_tok], F32, tag="lnr")
                            nc.scalar.activation(
                                lnr[:], up[64:65, 0:n_tok],
                                mybir.ActivationFunctionType.Ln)
                            rinv = p2s.tile([1, n_tok], mm_dt, tag="rinv")
                            nc.scalar.activation(
                                rinv[:], lnr[:],
                                mybir.ActivationFunctionType.Exp, scale=-1.0)
                            if usb_copy:
                                usb = p2s.tile([64, n_tok], F32, tag="usb")
                                nc.vector.tensor_copy(usb[:], up[0:64, 0:n_tok])
                                mul_in = usb[:]
                            else:
                                mul_in = up[0:64, 0:n_tok]
                            bc = psa.tile([128, 1024], F32, tag="psa")
                            for (s, e) in _nsplits(n_tok):
                                nc.tensor.matmul(
                                    bc[0:64, s:e], ones[0:1, 0:64],
                                    rinv[0:1, s:e], start=True, stop=True,
                                )
                            nc.vector.tensor_mul(
                                a_T[p0:p0 + 64, h // 2, :], mul_in,
                                bc[0:64, 0:n_tok])

                        for jt in range(nt):
                            dots(jt)
                            if fillers:
                                fillers.pop(0)()
                            workq.append(lambda jt=jt: pv(jt))
                            qstep()
                        while fillers:
                            fillers.pop(0)()
                        workq.append(norm)

                    def emit_pair_ab(pair):
                        # both heads of a pair interleaved per j-tile:
                        # alternating PE row groups overlap fill/drain, and
                        # exp feeds stay dense. PSUM: 2 dots tiles + 2 up
                        # accumulators = 8 banks.
                        qt = qk_tiles[pair]
                        kt_ = qk_tiles[ET // 2 + pair]
                        ups = {}
                        etsd = {0: {}, 1: {}}
                        for half in (0, 1):
                            ups[half] = pspv.tile([65, 1024], F32, tag="pv",
                                                  name=f"upab{half}")

                        def dots(half, jt):
                            p0 = half * 64
                            ps = psa.tile([128, 1024], F32, tag="psa")
                            for (s, e) in _nsplits(n_tok):
                                nc.tensor.matmul(
                                    ps[:, s:e],
                                    kt_[p0:p0 + 64, jt * 128:(jt + 1) * 128],
                                    qt[p0:p0 + 64, s:e],
                                    start=True, stop=True,
                                )
                            et = exps.tile([128, n_tok], mm_dt, tag="expS",
                                           name=f"etab{half}_{jt}")
                            nc.scalar.activation(
                                et[:], ps[:, 0:n_tok],
                                mybir.ActivationFunctionType.Exp, scale=SCALE)
                            etsd[half][jt] = et

                        def pv(half, jt):
                            h = 2 * pair + half
                            for (s, e) in _nsplits(n_tok):
                                nc.tensor.matmul(
                                    ups[half][:, s:e],
                                    v_aug[:, jt, h * 65:h * 65 + 65],
                                    etsd[half][jt][:, s:e],
                                    start=(jt == 0), stop=(jt == nt - 1),
                                )
                            del etsd[half][jt]

                        def norm(half):
                            h = 2 * pair + half
                            p0 = half * 64
                            up = ups[half]
                            lnr = p2s.tile([1, n_tok], F32, tag="lnr")
                            nc.scalar.activation(
                                lnr[:], up[64:65, 0:n_tok],
                                mybir.ActivationFunctionType.Ln)
                            rinv = p2s.tile([1, n_tok], mm_dt, tag="rinv")
                            nc.scalar.activation(
                                rinv[:], lnr[:],
                                mybir.ActivationFunctionType.Exp, scale=-1.0)
                            usb = p2s.tile([64, n_tok], F32, tag="usb")
                            nc.vector.tensor_copy(usb[:], up[0:64, 0:n_tok])
                            dst = a_T[p0:p0 + 64, h // 2, :]
                            if bcast_gpsimd:
                                # broadcast 1/r on the otherwise-idle GPSIMD,
                                # then multiply in place on DVE
                                nc.gpsimd.partition_broadcast(
                                    dst.bitcast(F32), rinv.bitcast(F32)[:])
                                nc.vector.tensor_mul(dst, dst.bitcast(F32),
                                                     usb[:])
                            else:
                                bc = psa.tile([128, 1024], F32, tag="psa")
                                for (s, e) in _nsplits(n_tok):
                                    nc.tensor.matmul(
                                        bc[0:64, s:e], ones[0:1, 0:64],
                                        rinv[0:1, s:e], start=True, stop=True,
                                    )
                                nc.vector.tensor_mul(
                                    dst, usb[:], bc[0:64, 0:n_tok])

                        ABLAG = ablag
                        for jt in range(nt):
                            dots(0, jt)
                            dots(1, jt)
                            if jt >= ABLAG:
                                pv(0, jt - ABLAG)
                                pv(1, jt - ABLAG)
                        for jt in range(max(nt - ABLAG, 0), nt):
                            pv(0, jt)
                            pv(1, jt)
                        if not skip_norm:
                            norm(0)
                            norm(1)

                    def emit_pair_fused(pair, fillers, lagi=6):
                        # both halves interleaved per j-tile (row-group
                        # alternation overlaps PE fill/drain) while keeping
                        # the fused filler structure. PSUM: 2 dots tiles +
                        # 2 up accumulators = 8 banks.
                        qt = qk_tiles[pair]
                        kt_ = qk_tiles[ET // 2 + pair]
                        ups = {}
                        for half in (0, 1):
                            ups[half] = pspv.tile([65, 1024], F32, tag="pv",
                                                  name=f"upf{half}")
                        ets = {}

                        def dots(half, jt):
                            p0 = half * 64
                            ps = psa.tile([128, 1024], F32, tag="psa")
                            for (s, e) in _nsplits(n_tok):
                                nc.tensor.matmul(
                                    ps[:, s:e],
                                    kt_[p0:p0 + 64, jt * 128:(jt + 1) * 128],
                                    qt[p0:p0 + 64, s:e],
                                    start=True, stop=True,
                                )
                            et = exps.tile([128, n_tok], mm_dt, tag="expS",
                                           name=f"etf{half}_{jt}")
                            nc.scalar.activation(
                                et[:], ps[:, 0:n_tok],
                                mybir.ActivationFunctionType.Exp, scale=SCALE)
                            ets[(half, jt)] = et

                        def pv(half, jt):
                            h = 2 * pair + half
                            for (s, e) in _nsplits(n_tok):
                                nc.tensor.matmul(
                                    ups[half][:, s:e],
                                    v_aug[:, jt, h * 65:h * 65 + 65],
                                    ets[(half, jt)][:, s:e],
                                    start=(jt == 0), stop=(jt == nt - 1),
                                )
                            del ets[(half, jt)]

                        rinvs = {}
                        usbs = {}

                        def norm_act(half):
                            up = ups[half]
                            if norm_pool:
                                # copy out^T + r row to SBUF (frees the PSUM
                                # bank in one op), 1/r approx on DVE: no ACT
                                # table pressure, no [1,n] ln/exp lane waste
                                usb = p2s.tile([65, n_tok], F32,
                                               tag=f"usb{half}",
                                               name=f"usbf{half}")
                                nc.vector.tensor_copy(usb[:],
                                                      up[0:65, 0:n_tok])
                                rinv = p2s.tile([1, n_tok], F32,
                                                tag=f"rinv{half}",
                                                name=f"rinvf{half}")
                                if norm_pool == 3:
                                    lnr = p2s.tile([1, n_tok], F32,
                                                   tag=f"lnr{half}")
                                    nc.scalar.activation(
                                        lnr[:], usb[64:65, :],
                                        mybir.ActivationFunctionType.Ln)
                                    nc.scalar.activation(
                                        rinv[:], lnr[:],
                                        mybir.ActivationFunctionType.Exp,
                                        scale=-1.0)
                                else:
                                    # custom DVE / gpsimd ISA ops read the
                                    # memloc's partition 0 regardless of the
                                    # AP offset: stage the r row (partition
                                    # 64) to a partition-0 tile via an
                                    # SBUF->SBUF DMA (off-engine, idle in
                                    # phase 2) before the DVE reciprocal
                                    rrow = p2s.tile([1, n_tok], F32,
                                                    tag=f"rrow{half}",
                                                    name=f"rrowf{half}")
                                    nc.sync.dma_start(rrow[:],
                                                      usb[64:65, :])
                                    nc.vector.reciprocal_approx_fast(
                                        rinv[:], rrow[:])
                                usbs[half] = usb
                                rinvs[half] = rinv
                                return
                            lnr = p2s.tile([1, n_tok], F32, tag="lnr")
                            nc.scalar.activation(
                                lnr[:], up[64:65, 0:n_tok],
                                mybir.ActivationFunctionType.Ln)
                            rinv = p2s.tile([1, n_tok], mm_dt,
                                            tag=(f"rinv{half}" if norm_split
                                                 else "rinv"),
                                            name=f"rinvf{half}")
                            nc.scalar.activation(
                                rinv[:], lnr[:],
                                mybir.ActivationFunctionType.Exp, scale=-1.0)
                            rinvs[half] = rinv

                        def norm_rest(half):
                            h = 2 * pair + half
                            p0 = half * 64
                            up = ups[half]
                            rinv = rinvs[half]
                            if norm_pool:
                                # broadcast 1/r on the idle Pool/GPSIMD,
                                # multiply on DVE
                                usb = usbs[half]
                                if norm_pool == 2:
                                    rin16 = p2s.tile([1, n_tok], mm_dt,
                                                     tag=f"ri16{half}")
                                    nc.vector.tensor_copy(rin16[:], rinv[:])
                                    bcp = psa.tile([128, 1024], F32,
                                                   tag="psa")
                                    for (s, e) in _nsplits(n_tok):
                                        nc.tensor.matmul(
                                            bcp[0:64, s:e], ones[0:1, 0:64],
                                            rin16[0:1, s:e],
                                            start=True, stop=True)
                                    nc.vector.tensor_mul(
                                        a_T[p0:p0 + 64, h // 2, :],
                                        usb[0:64, :], bcp[0:64, 0:n_tok])
                                    return
                                bc = p2s.tile([64, n_tok], F32,
                                              tag=f"bc{half}",
                                              name=f"bcf{half}")
                                nc.gpsimd.partition_broadcast(bc[:], rinv[:])
                                nc.vector.tensor_mul(
                                    a_T[p0:p0 + 64, h // 2, :],
                                    usb[0:64, :], bc[:])
                                return
                            usb = p2s.tile([64, n_tok], F32, tag="usb")
                            nc.vector.tensor_copy(usb[:], up[0:64, 0:n_tok])
                            bc = psa.tile([128, 1024], F32, tag="psa")
                            for (s, e) in _nsplits(n_tok):
                                nc.tensor.matmul(
                                    bc[0:64, s:e], ones[0:1, 0:64],
                                    rinv[0:1, s:e], start=True, stop=True,
                                )
                            nc.vector.tensor_mul(
                                a_T[p0:p0 + 64, h // 2, :], usb[:],
                                bc[0:64, 0:n_tok])

                        for jt in range(nt):
                            dots(0, jt)
                            dots(1, jt)
                            if fillers:
                                fillers.pop(0)()
                            workq.append(lambda jt=jt: pv(0, jt))
                            workq.append(lambda jt=jt: pv(1, jt))
                            while len(workq) > lagi:
                                workq.pop(0)()
                        while fillers:
                            fillers.pop(0)()
                        if norm_split:
                            workq.append(lambda: norm_act(0))
                            workq.append(lambda: norm_act(1))
                            workq.append(lambda: norm_rest(0))
                            workq.append(lambda: norm_rest(1))
                        else:
                            workq.append(lambda: (norm_act(0), norm_rest(0)))
                            workq.append(lambda: (norm_act(1), norm_rest(1)))

                    # software-pipelined emission driver
                    if 1 in phases and use2:
                        emit_1a_pair(0)
                        emit_1a_pair(1)
                        # filler queues: 1b chains ride inside pair 0; 1a
                        # prefetch for pair pr rides inside pair pr-2, half B
                        fill = {}
                        for pair in range(npairs):
                            for half in (0, 1):
                                fill[(pair, half)] = []
                        for it in range(nt):
                            u = (0, 0) if it < 6 else (0, 1)
                            fill[u].append(lambda it=it: emit_1b_one(it))
                        for pr in range(2, ET // 2):
                            host = (pr - 2, 1)
                            if host not in fill:
                                host = (npairs - 1, 1)
                            fill[host].append(lambda m=pr: emit_1a_one(m))
                            fill[host].append(
                                lambda m=ET // 2 + pr: emit_1a_one(m))
                        if fused_ab:
                            if fill_even:
                                # re-spread: 1b 0..5 in pair 0; 1b 6,7 +
                                # 1a(2) in pair 1; 1a(pr) in pair pr-2
                                fl = {p: [] for p in range(npairs)}
                                fl[0] = [lambda it=it: emit_1b_one(it)
                                         for it in range(min(6, nt))]
                                if npairs > 1:
                                    fl[1] = ([lambda it=it: emit_1b_one(it)
                                              for it in range(6, nt)] +
                                             [lambda: emit_1a_one(2),
                                              lambda: emit_1a_one(ET // 2 + 2)])
                                for pr in range(3, ET // 2):
                                    host = min(pr - 2, npairs - 1)
                                    fl[host].append(
                                        lambda m=pr: emit_1a_one(m))
                                    fl[host].append(
                                        lambda m=ET // 2 + pr: emit_1a_one(m))
                                for pair in range(npairs):
                                    emit_pair_fused(pair, fl[pair], lagi=lagi)
                            else:
                                for pair in range(npairs):
                                    emit_pair_fused(
                                        pair,
                                        fill[(pair, 0)] + fill[(pair, 1)],
                                        lagi=lagi)
                        else:
                            for pair in range(npairs):
                                for half in (0, 1):
                                    emit_unit(pair, half, fill[(pair, half)])
                        qstep(force=True)
                    elif 1 in phases:
                        emit_1b()
                        for pr in range(ET // 2):
                            emit_1a_pair(pr)
                    else:
                        for pair in range(npairs):
                            for half in (0, 1):
                                emit_unit(pair, half, [])
                        qstep(force=True)

                # ------------- phase 3: output projection + bias -------------
                with (
                    tc.tile_pool(name="pw", bufs=1) as pw,
                    tc.tile_pool(name="p3o", bufs=3) as p3o,
                ):
                    if 3 in phases:
                        wout = wout_sb
                        if 1 not in phases:
                            for kt in range(ATT):
                                nc.sync.dma_start(
                                    wout[:, kt, :],
                                    wout_d.ap()[kt * 128:(kt + 1) * 128, :])
                        if bias_dve:
                            # bias broadcast built once; the i-tile loop adds
                            # it on the otherwise-idle DVE instead of 16 K=1
                            # matmuls on the (binding) PE
                            bias_bc = pw.tile([128, DIM], F32)
                            bps = psa.tile([128, 1024], F32, tag="psa")
                            for (s, e) in _nsplits(DIM):
                                nc.tensor.matmul(
                                    bps[0:128, s:e], ones[0:1, 0:128],
                                    bout[0:1, s:e], start=True, stop=True,
                                )
                            nc.scalar.copy(bias_bc[:], bps[:, 0:DIM])
                        for it in range(nt):
                            if p3_deep and it % 2 == 1:
                                # the PV accumulator banks are idle in p3:
                                # alternate output accumulators across both
                                # pools for a deeper pipeline
                                po = pspv.tile([128, 1024], F32, tag="pv",
                                               name="po_b")
                            else:
                                po = psa.tile([128, 1024], F32, tag="psa")
                            for kt in range(ATT):
                                for (s, e) in _nsplits(DIM):
                                    nc.tensor.matmul(
                                        po[:, s:e],
                                        a_T[:, kt, it * 128:(it + 1) * 128],
                                        wout[:, kt, s:e],
                                        start=(kt == 0),
                                        stop=(bias_dve and kt == ATT - 1),
                                    )
                            if not bias_dve:
                                for (s, e) in _nsplits(DIM):
                                    nc.tensor.matmul(
                                        po[:, s:e], ones[0:1, 0:128],
                                        bout[0:1, s:e],
                                        start=False, stop=True,
                                    )
                            osb = p3o.tile([128, DIM], mm_dt, tag="osb")
                            if bias_dve:
                                nc.vector.tensor_add(osb[:], po[:, 0:DIM],
                                                     bias_bc[:])
                            else:
                                nc.scalar.copy(osb[:], po[:, 0:DIM])
                            nc.sync.dma_start(
                                o_d.ap()[it * 128:(it + 1) * 128, :], osb[:])


    nc.compile()
    return nc


def host_prep(x, w_qkv, w_out, b_out, batch=BATCH):
    """Build per-core input maps from the full problem inputs."""
    x = np.asarray(x, dtype=np.float32)
    w_qkv = np.asarray(w_qkv, dtype=np.float32)
    w_out = np.asarray(w_out, dtype=np.float32)
    b_out = np.asarray(b_out, dtype=np.float32)

    w_q = w_qkv[:, 0:INNER]
    w_k = w_qkv[:, INNER:2 * INNER]
    w_v = w_qkv[:, 2 * INNER:3 * INNER]
    wqk = np.zeros((DIM, 2 * INNER), dtype=np.float32)
    for p in range(HEADS // 2):
        wqk[:, p * 256:p * 256 + 128] = w_q[:, p * 128:(p + 1) * 128]
        wqk[:, p * 256 + 128:(p + 1) * 256] = w_k[:, p * 128:(p + 1) * 128]
    wv = np.zeros((DIM, VW), dtype=np.float32)
    for h in range(HEADS):
        wv[:, h * 65:h * 65 + 64] = w_v[:, h * 64:(h + 1) * 64]
    hd = np.float16  # matmul operand dtype on device (must match MM_DT)
    shared = {
        "ones": np.ones((1, 128), dtype=hd),
        "wqk": wqk.astype(hd),
        "wv": wv.astype(hd),
        "wout": np.ascontiguousarray(w_out).astype(hd),
        "bout": np.ascontiguousarray(b_out.reshape(1, DIM)).astype(hd),
    }
    in_maps = []
    for b in range(batch):
        m = dict(shared)
        m["xT"] = np.ascontiguousarray(x[b].T).astype(hd)
        in_maps.append(m)
    return in_maps


# --- inline PJRT runner (build once, call many) ---
def _make_runner(nc, n_cores):
    import jax
    from jax.sharding import Mesh, PartitionSpec
    from jax.experimental.shard_map import shard_map
    from concourse import bass2jax

    bass2jax.install_neuronx_cc_hook()
    partition_name = nc.partition_id_tensor.name if nc.partition_id_tensor else None
    in_names, out_names, out_avals, zero_outs = [], [], [], []
    for alloc in nc.m.functions[0].allocations:
        if not isinstance(alloc, mybir.MemoryLocationSet):
            continue
        name = alloc.memorylocations[0].name
        if alloc.kind == "ExternalInput":
            if name != partition_name:
                in_names.append(name)
        elif alloc.kind == "ExternalOutput":
            shape = tuple(alloc.tensor_shape)
            dtype = mybir.dt.np(alloc.dtype)
            out_names.append(name)
            out_avals.append(jax.core.ShapedArray(shape, dtype))
            zero_outs.append(np.zeros(shape, dtype))
    n_params = len(in_names)
    n_outs = len(out_avals)
    all_in_names = list(in_names) + list(out_names)
    if partition_name is not None:
        all_in_names.append(partition_name)

    def _body(*args):
        operands = list(args)
        if partition_name is not None:
            operands.append(bass2jax.partition_id_tensor())
        outs = bass2jax._bass_exec_p.bind(
            *operands,
            out_avals=tuple(out_avals),
            in_names=tuple(all_in_names),
            out_names=tuple(out_names),
            lowering_input_output_aliases=(),
            sim_require_finite=True,
            sim_require_nnan=True,
            nc=nc,
        )
        return tuple(outs)

    donate = tuple(range(n_params, n_params + n_outs))
    if n_cores == 1:
        fn = jax.jit(_body, donate_argnums=donate, keep_unused=True)

        def run(in_maps):
            args = [np.asarray(in_maps[0][n]) for n in in_names]
            out_arrs = fn(*args, *[z.copy() for z in zero_outs])
            jax.block_until_ready(out_arrs)
            return [{n: np.asarray(out_arrs[i]) for i, n in enumerate(out_names)}]
        return run

    devices = jax.devices()[:n_cores]
    mesh = Mesh(np.asarray(devices), ("core",))
    in_specs = (PartitionSpec("core"),) * (n_params + n_outs)
    out_specs = (PartitionSpec("core"),) * n_outs
    fn = jax.jit(
        shard_map(_body, mesh=mesh, in_specs=in_specs, out_specs=out_specs,
                  check_rep=False),
        donate_argnums=donate, keep_unused=True,
    )

    def run(in_maps):
        per_core = [[np.asarray(m[n]) for n in in_names] for m in in_maps]
        concat_in = [
            np.concatenate([per_core[c][i] for c in range(n_cores)], axis=0)
            for i in range(n_params)
        ]
        concat_zeros = [
            np.zeros((n_cores * z.shape[0], *z.shape[1:]), z.dtype)
            for z in zero_outs
        ]
        out_arrs = fn(*concat_in, *concat_zeros)
        jax.block_until_ready(out_arrs)
        return [
            {n: np.asarray(out_arrs[i]).reshape(n_cores, *out_avals[i].shape)[c]
             for i, n in enumerate(out_names)}
            for c in range(n_cores)
        ]
    return run


_CACHE = {}


def get_runner():
    if "run" not in _CACHE:
        nc = build_nc()
        _CACHE["nc"] = nc
        _CACHE["run"] = _make_runner(nc, N_CORES)
    return _CACHE["run"]


def kernel(x, w_qkv, w_out, b_out):
    run = get_runner()
    in_maps = host_prep(x, w_qkv, w_out, b_out)
    res = run(in_maps)
    return np.stack([res[b]["o"] for b in range(BATCH)],
                    axis=0).astype(np.float32)

